# revision 19
# baseline (speedup 1.0000x reference)
"""CrossDomainAttention TRN2 kernel: 8-core data-parallel over batch.

Reference computation (per batch element, a/b are (L, C) slices):
  ap = a.T (C, L);  q = ap@Wq.T+bq; k,v from b.T
  attn = softmax(q @ k.T / sqrt(L)) (C, C)
  out = LN(attn @ v + ap) over L, returned as the raw (C*L) buffer viewed (L, C)

v6: fp8 (e4m3) DoubleRow with a j-granular 2-deep software pipeline.
Work is cut into 16 "units" (elem x c-chunk-pair x j); each unit's
rounds interleave on the PE: scores(unit, dp) [4 DR mms + 1 exp on Act]
with half-PV-chains of the previous unit and filler blocks (next
element's DMA/casts/transposes/projections), keeping the PE dense so
HAM stays warm and the drain tail is a single unit's 4 PV chains.
Row-sums are ones-lhsT matmuls at unit end (PSUM in the PV pool),
transposed to per-partition columns via tiny PE transposes.  LN uses
the scale-invariant form (out_pre = rowsum*apT + PV) with a batched
Newton rsqrt.  Residual apT is fp16, transposed from an fp16 copy of a
at 1 cyc/row.
"""

import numpy as np

B, L, C = 16, 512, 2048
NCORE = 8
NB = B // NCORE          # batch elements per core
P = 128
F = 512                  # matmul free-dim tile
NLC = L // P             # 4  l/m chunks
NDB = C // P             # 16 d-blocks / c-blocks
NCCH = C // F            # 4  c chunks
NDP = NDB // 2           # 8  d-pairs (DoubleRow)
NLP = NLC // 2           # 2  l/m pairs (DoubleRow)
LN_EPS = 1e-5
RSTD_SEED = 4.77e-4   # ~1/sqrt(mean var') for the scale-invariant LN form
INV_SQRT_L = 1.0 / float(np.sqrt(L))

_CACHE = {}


def _build(apply_qkv_bias: bool, apply_gamma_beta: bool, repeat: int = 1):
    import concourse.bass as bass
    import concourse.tile as tile
    from concourse import bacc, mybir
    from concourse.bass import ts, ds
    from concourse.masks import make_identity
    from contextlib import ExitStack

    f32 = mybir.dt.float32
    f16 = mybir.dt.float16
    f8 = mybir.dt.float8e4
    AF = mybir.ActivationFunctionType
    ALU = mybir.AluOpType
    DR = mybir.MatmulPerfMode.DoubleRow

    nc = bacc.Bacc("TRN2", target_bir_lowering=False, debug=False,
                   enable_asserts=False)

    a_d = nc.dram_tensor("a", (NB, L, C), f32, kind="ExternalInput").ap()
    b_d = nc.dram_tensor("b", (NB, L, C), f32, kind="ExternalInput").ap()
    w_d = {n: nc.dram_tensor(n, (L, L), f32, kind="ExternalInput").ap()
           for n in ("Wq", "Wk", "Wv")}
    bias_d = {n: nc.dram_tensor(n, (L,), f32, kind="ExternalInput").ap()
              for n in ("bq", "bk", "bv")}
    gamma_d = nc.dram_tensor("gamma", (L,), f32, kind="ExternalInput").ap()
    beta_d = nc.dram_tensor("beta", (L,), f32, kind="ExternalInput").ap()
    out_d = nc.dram_tensor("out", (NB, C, L), f32, kind="ExternalOutput").ap()

    def bcast_p(ap1d):
        return bass.AP(tensor=ap1d.tensor, offset=ap1d.offset,
                       ap=[[0, P]] + [list(d) for d in ap1d.ap])

    ELEMS = [i % NB for i in range(NB * repeat)]

    with tile.TileContext(nc) as tc, ExitStack() as ctx:
        const = ctx.enter_context(tc.tile_pool(name="const", bufs=1))
        ld = ctx.enter_context(tc.tile_pool(name="ld", bufs=2))
        c16 = ctx.enter_context(tc.tile_pool(name="c16", bufs=2))
        epool = ctx.enter_context(tc.tile_pool(name="epool", bufs=2))
        pt = ctx.enter_context(tc.tile_pool(name="pt", bufs=4))
        outp = ctx.enter_context(tc.tile_pool(name="outp", bufs=6))
        small = ctx.enter_context(tc.tile_pool(name="small", bufs=2))
        # PSUM: mm 3x[P,2,F](6 banks) + pv 2x[P,F](2) = 8.  The scores
        # pool is 3 deep so exp (Act) never gates the PE; psr lives in an
        # mm slot, trs in a pv slot.
        ps_mm = ctx.enter_context(tc.tile_pool(name="ps_mm", bufs=3, space="PSUM"))
        ps_pv = ctx.enter_context(tc.tile_pool(name="ps_pv", bufs=2, space="PSUM"))

        def cp(e, dst, src):
            if e is nc.scalar:
                e.copy(dst, src)
            else:
                e.tensor_copy(dst, src)

        # ---- constants ----
        ident16 = const.tile([P, P], f16, tag="ident16")
        make_identity(nc, ident16)
        ident1 = const.tile([1, 1], f32, tag="ident1")
        nc.vector.memset(ident1[:], 1.0)
        ones2 = const.tile([P, 2, 16], f8, tag="ones2")
        nc.vector.memset(ones2[:], 1.0)
        bias_col = {}
        bv_bc = None
        if apply_qkv_bias:
            cpack = const.tile([P, 16], f32, tag="cpack")
            for i, n in enumerate(("bq", "bk")):
                dst = cpack[:, 4 * i: 4 * (i + 1)]
                nc.sync.dma_start(dst, bias_d[n].rearrange("(o p) -> p o", p=P))
                bias_col[n] = dst
            bv_bc = const.tile([P, L], f32, tag="bv_bc")
            nc.sync.dma_start(bv_bc[:], bcast_p(bias_d["bv"]))
        if apply_gamma_beta:
            gb_pack = const.tile([P, 2, L], f32, tag="gb")
            nc.sync.dma_start(gb_pack[:, 0, :], bcast_p(gamma_d))
            nc.sync.dma_start(gb_pack[:, 1, :], bcast_p(beta_d))

        WT = {n: const.tile([P, NLC, L], f8, tag=f"WT_{n}", name=f"WT_{n}")
              for n in ("Wq", "Wk", "Wv")}

        # ---------- emission helpers ----------
        w_eng = [nc.vector, nc.scalar]

        def load_weight_blocks(n, wi):
            def emit():
                wld = ld.tile([P, NLC, F], f32, tag="ld", name=f"wld_{n}")
                nc.sync.dma_start(wld[:],
                                  w_d[n].rearrange("(o p) l -> p o l", p=P))
                w16 = c16.tile([P, NLC, F], f16, tag="w16", bufs=1,
                               name=f"w16_{n}")
                nc.vector.tensor_copy(w16[:], wld[:])
                for li in range(NLC):
                    pst = ps_mm.tile([P, F], f16, tag="mm", name="wtp")
                    for mo in range(NLC):
                        nc.tensor.transpose(pst[:, ts(mo, P)],
                                            w16[:, mo, ts(li, P)], ident16[:])
                    cp(w_eng[(wi + li) % 2], WT[n][:, li, :], pst[:])
            return [emit]

        class E:
            pass

        def make_elem(bi, rep):
            e = E()
            e.bi = bi
            e.rep = rep
            e.a8 = epool.tile([P, NLC, C], f8, tag="a8", name=f"a8_{rep}")
            e.b8 = epool.tile([P, NLC, C], f8, tag="b8", name=f"b8_{rep}")
            e.qT = epool.tile([P, NLC, C], f8, tag="qT", name=f"qT_{rep}")
            e.kT = epool.tile([P, NLC, C], f8, tag="kT", name=f"kT_{rep}")
            e.v8 = epool.tile([P, NDB, L], f8, tag="v8", name=f"v8_{rep}")
            e.apT = epool.tile([P, NDB, L], f16, tag="apT", name=f"apT_{rep}")
            e.a16 = {}
            return e

        a_cast8 = [nc.vector, nc.scalar, nc.vector, nc.scalar]
        b_cast8 = [nc.scalar, nc.vector, nc.scalar, nc.vector]
        apt_cp = [nc.vector, nc.scalar, nc.vector, nc.scalar]

        def load_a_chunk(e, li):
            def emit():
                ach = ld.tile([P, C], f32, tag="ld", name=f"a_{e.rep}_{li}")
                nc.sync.dma_start(ach[:], a_d[e.bi, ds(li * P, P), :])
                a16 = c16.tile([P, C], f16, tag="a16", bufs=4,
                               name=f"a16_{e.rep}_{li}")
                nc.vector.tensor_copy(a16[:], ach[:])
                cp(a_cast8[li], e.a8[:, li, :], ach[:])
                e.a16[li] = a16
            return [emit]

        def apt_blocks(e, li):
            # 2 transposes per sub-block; copy after the 2nd sub-block
            state = {}

            def mk(g, h):
                def emit():
                    a16 = e.a16[li]
                    if h == 0:
                        state[g] = ps_mm.tile([P, F], f16, tag="mm",
                                              name="atp")
                    pst = state[g]
                    for j in range(2):
                        db = 4 * g + 2 * h + j
                        nc.tensor.transpose(pst[:, ts(2 * h + j, P)],
                                            a16[:, ts(db, P)], ident16[:])
                    if h == 1:
                        dst = e.apT[:, 4 * g:4 * g + 4, ts(li, P)]
                        cp(apt_cp[(li + g) % 2],
                           dst, pst[:].rearrange("p (b f) -> p b f", f=P))
                return emit
            return [mk(g, h) for g in range(NLC) for h in range(2)]

        def load_b_split(e):
            tiles = {}

            def dma_blk():
                for li in range(NLC):
                    bch = ld.tile([P, C], f32, tag="ld",
                                  name=f"b_{e.rep}_{li}")
                    nc.sync.dma_start(bch[:], b_d[e.bi, ds(li * P, P), :])
                    tiles[li] = bch

            def mk_cast(li):
                def emit():
                    cp(b_cast8[li], e.b8[:, li, :], tiles[li])
                return emit
            return [dma_blk], [mk_cast(li) for li in range(NLC)]

        qk_cp = [nc.vector, nc.scalar]

        def qk_proj_blocks(e, wname, bname, src_name, dst_name):
            def mk(mi):
                def emit():
                    src = getattr(e, src_name)
                    dst = getattr(e, dst_name)
                    pss = [ps_mm.tile([P, F], f32, tag="mm", name=f"qk{i}")
                           for i in range(4)]
                    for lp in range(NLP):
                        for ci in range(NCCH):
                            nc.tensor.matmul(
                                pss[ci][:],
                                lhsT=WT[wname][:, 2 * lp:2 * lp + 2, ts(mi, P)],
                                rhs=src[:, 2 * lp:2 * lp + 2, ts(ci, F)],
                                start=(lp == 0), stop=(lp == NLP - 1),
                                perf_mode=DR)
                    for ci in range(NCCH):
                        dslice = dst[:, mi, ts(ci, F)]
                        if apply_qkv_bias:
                            nc.scalar.activation(
                                dslice, pss[ci][:], AF.Identity,
                                bias=bias_col[bname][:, mi:mi + 1])
                        else:
                            cp(qk_cp[ci % 2], dslice, pss[ci][:])
                return emit
            return [mk(mi) for mi in range(NLC)]

        def v_proj_blocks(e):
            def mk(dp):
                def emit():
                    pss = [ps_mm.tile([P, F], f32, tag="mm", name=f"v{i}")
                           for i in range(2)]
                    for s in range(2):
                        di = 2 * dp + s
                        for lp in range(NLP):
                            nc.tensor.matmul(
                                pss[s][:],
                                lhsT=e.b8[:, 2 * lp:2 * lp + 2, ts(di, P)],
                                rhs=WT["Wv"][:, 2 * lp:2 * lp + 2, :],
                                start=(lp == 0), stop=(lp == NLP - 1),
                                perf_mode=DR)
                    for s in range(2):
                        cp(qk_cp[(dp + s) % 2], e.v8[:, 2 * dp + s, :],
                           pss[s][:])
                        if apply_qkv_bias:
                            nc.vector.tensor_add(e.v8[:, 2 * dp + s, :],
                                                 e.v8[:, 2 * dp + s, :],
                                                 bv_bc[:, :])
                return emit
            return [mk(dp) for dp in range(NDP)]

        # ---------- attention units ----------

        class Unit:
            # one (elem, p, j): a 1024-row slab of the attention output
            def __init__(u, e, p, j):
                u.e, u.p, u.j = e, p, j
                u.PT = pt.tile([P, NDB, F], f8, tag="pt",
                               name=f"pt_{e.rep}_{p}_{j}")
                u.psr = None
                u.rs_post = None
                u.rs_cols = None
                u.stats = small.tile([P, NCCH, 2], f32, tag="stats",
                                     name=f"st_{e.rep}_{p}_{j}")
                u.chain = {}
                u.outs = {}

        def scores_round(u, dp):
            # 4 DR mms -> [P,2,F] psum; 1 exp (Act)
            def emit():
                e = u.e
                pss = ps_mm.tile([P, 2, F], f32, tag="mm", name="sc")
                for sb in range(2):
                    di = 2 * dp + sb
                    for mp in range(NLP):
                        nc.tensor.matmul(
                            pss[:, sb, :],
                            lhsT=e.kT[:, 2 * mp:2 * mp + 2, ts(di, P)],
                            rhs=e.qT[:, 2 * mp:2 * mp + 2,
                                     ts(2 * u.p + u.j, F)],
                            start=(mp == 0), stop=(mp == NLP - 1),
                            perf_mode=DR)
                nc.scalar.activation(u.PT[:, 2 * dp:2 * dp + 2, :],
                                     pss[:], AF.Exp, scale=INV_SQRT_L)
            return emit

        def rs_block(u):
            # 8 N=512 ones-lhsT mms -> psr row; psrow copy (Act); 4 tiny
            # transposes; copy columns to SBUF.
            def emit_mms():
                u.psr = ps_mm.tile([16, F], f32, tag="mm", name="psr")
                for dp in range(NDP):
                    nc.tensor.matmul(u.psr[:],
                                     lhsT=ones2[:],
                                     rhs=u.PT[:, 2 * dp:2 * dp + 2, :],
                                     start=(dp == 0), stop=(dp == NDP - 1),
                                     perf_mode=DR,
                                     skip_group_check=True)

            def emit_post():
                psrow = small.tile([1, F], f32, tag="rrow", name="psrow")
                nc.scalar.copy(psrow[:], u.psr[0:1, :])
                trs_ps = ps_pv.tile([P, NCCH], f32, tag="pv", name="trs")
                for cb in range(NCCH):
                    nc.tensor.transpose(trs_ps[:, cb:cb + 1],
                                        psrow[0:1, ts(cb, P)],
                                        ident1[:])
                u.rs_cols = small.tile([P, NCCH], f32, tag="rcol",
                                       name="rs_cols")
                nc.vector.tensor_copy(u.rs_cols[:], trs_ps[:])
            return [emit_mms, emit_post]

        def pv_half(u, cb, h):
            # half of one PV chain: 4 DR mms; on h==1 also stt+bn (DVE)
            def emit():
                e = u.e
                if h == 0:
                    u.chain[cb] = ps_pv.tile([P, L], f32, tag="pv",
                                             name="po")
                po = u.chain[cb]
                for dp in range(4 * h, 4 * h + 4):
                    nc.tensor.matmul(
                        po[:],
                        lhsT=u.PT[:, 2 * dp:2 * dp + 2, ts(cb, P)],
                        rhs=e.v8[:, 2 * dp:2 * dp + 2, :],
                        start=(dp == 0), stop=(dp == NDP - 1),
                        perf_mode=DR)
                if h == 1:
                    gb = (2 * u.p + u.j) * NCCH + cb
                    out_sb = outp.tile([P, L], f16, tag="out",
                                       name="out_sb")
                    nc.vector.scalar_tensor_tensor(
                        out_sb[:], e.apT[:, gb, :],
                        u.rs_cols[:, cb:cb + 1], po[:], ALU.mult, ALU.add)
                    st6 = small.tile([P, 6], f32, tag="st6", name="st6")
                    nc.vector.bn_stats(st6[:], out_sb[:])
                    nc.vector.bn_aggr(u.stats[:, cb, :], st6[:])
                    u.outs[cb] = out_sb
            return emit

        def ln_finish(u, cb0=0, ncb=NCCH):
            # batched Newton rsqrt over ncb cb's; final scale + DMA
            def emit():
                e = u.e
                var_ap = u.stats[:, cb0:cb0 + ncb, 1]
                y = small.tile([P, 2, NCCH], f32, tag="nwt", name="nwt")
                y = y[:, :, :ncb]
                nc.vector.tensor_scalar(y[:, 0, :], var_ap,
                                        -0.5 * RSTD_SEED ** 3,
                                        1.5 * RSTD_SEED,
                                        ALU.mult, ALU.add)
                for _ in range(3):
                    t = y[:, 1, :]
                    nc.vector.tensor_mul(t, y[:, 0, :], y[:, 0, :])
                    nc.vector.tensor_mul(t, t, var_ap)
                    nc.vector.tensor_scalar(t, t, -0.5, 1.5,
                                            ALU.mult, ALU.add)
                    nc.vector.tensor_mul(y[:, 0, :], y[:, 0, :], t)
                for i, cb in enumerate(range(cb0, cb0 + ncb)):
                    gb = (2 * u.p + u.j) * NCCH + cb
                    out_sb = u.outs[cb]
                    out32 = outp.tile([P, L], f32, tag="out32",
                                      bufs=3, name="out32")
                    nc.vector.tensor_scalar(out32[:], out_sb[:],
                                            u.stats[:, cb, 0:1],
                                            y[:, 0, i:i + 1],
                                            ALU.subtract, ALU.mult)
                    if apply_gamma_beta:
                        nc.vector.tensor_mul(out32[:], out32[:],
                                             gb_pack[:, 0, :])
                        nc.vector.tensor_add(out32[:], out32[:],
                                             gb_pack[:, 1, :])
                    nc.sync.dma_start(out_d[e.bi, ds(gb * P, P), :],
                                      out32[:])
            return emit

        def unit_rounds(u, prev_unit, prevs, fillers):
            # 8 rounds of: scores(dp) + prev half-chains + fillers
            halves = []
            for pu in prevs:
                for cb in range(NCCH):
                    halves.append(pv_half(pu, cb, 0))
                    halves.append(pv_half(pu, cb, 1))
            per_round = (len(halves) + NDP - 1) // NDP if halves else 0
            blocks = []
            hi = 0
            for dp in range(NDP):
                blocks.append(scores_round(u, dp))
                for _ in range(per_round):
                    if hi < len(halves):
                        blocks.append(halves[hi])
                        hi += 1
                if dp == 0 and prev_unit is not None:
                    blocks.append(prev_unit.rs_post)
                if fillers:
                    blocks.append(fillers.pop(0))
                    if len(fillers) > 2 * (NDP - 1 - dp):
                        blocks.append(fillers.pop(0))
            while hi < len(halves):
                blocks.append(halves[hi])
                hi += 1
            rsb = rs_block(u)
            blocks.append(rsb[0])
            u.rs_post = rsb[1]
            for pu in prevs:
                blocks.append(ln_finish(pu))
            while fillers:
                blocks.append(fillers.pop(0))
            return blocks

        def drain_unit(u):
            blocks = [u.rs_post]
            for cb in range(NCCH):
                blocks.append(pv_half(u, cb, 0))
                blocks.append(pv_half(u, cb, 1))
                if cb % 2 == 1:
                    blocks.append(ln_finish(u, cb - 1, 2))
            return blocks

        # ---------- schedule ----------
        elems = [make_elem(bi, rep) for rep, bi in enumerate(ELEMS)]

        def load_a_blocks(e):
            return [blk for li in range(NLC) for blk in load_a_chunk(e, li)]

        def elem_apt_blocks(e):
            return [blk for li in range(NLC) for blk in apt_blocks(e, li)]

        # head: critical path Wq+Wk+a0+b0 DMA -> q,k proj -> scores.
        e0 = elems[0]
        la = load_a_blocks(e0)
        b_dma0, b_casts0 = load_b_split(e0)
        head = []
        head += load_weight_blocks("Wq", 0)
        head += [la[0], la[1]]
        head += load_weight_blocks("Wk", 1)
        head += [la[2], la[3]]
        head += b_dma0
        head += qk_proj_blocks(e0, "Wq", "bq", "a8", "qT")
        head += elem_apt_blocks(e0)
        head += b_casts0
        head += qk_proj_blocks(e0, "Wk", "bk", "b8", "kT")
        if len(elems) > 1:
            head += load_a_blocks(elems[1])
        for blk in head:
            blk()

        # unit pipeline.  fillers[ui] rides unit ui's rounds.
        units = []
        for e in elems:
            for p in range(2):
                for j in range(2):
                    units.append((e, p, j))
        NU = len(units)
        fillers = [[] for _ in range(NU)]
        # elem0's v-proj (Wv lands after b0) rides unit 1; unit 0's PV
        # is deferred one unit.
        fillers[1] = load_weight_blocks("Wv", 0) + v_proj_blocks(e0)
        for ei in range(len(elems) - 1):
            en = elems[ei + 1]
            u0 = 4 * ei
            apt = elem_apt_blocks(en)
            b_dma, b_casts = load_b_split(en)
            qb = qk_proj_blocks(en, "Wq", "bq", "a8", "qT")
            kb = qk_proj_blocks(en, "Wk", "bk", "b8", "kT")
            if ei > 0:
                fillers[u0 + 0] += load_a_blocks(en)
            fillers[u0 + 0] += b_dma + apt[:8]
            fillers[u0 + 1] += apt[8:24]
            fillers[u0 + 2] += apt[24:] + qb[:2]
            fillers[u0 + 3] += qb[2:] + b_casts + kb
            fillers[u0 + 4] += v_proj_blocks(en)

        prevs_list = [[] for _ in range(NU)]
        for ui in range(2, NU):
            prevs_list[ui] = [0, 1] if ui == 2 else [ui - 1]

        unit_objs = []
        for ui, (e, p, j) in enumerate(units):
            u = Unit(e, p, j)
            unit_objs.append(u)
            prevs = [unit_objs[k] for k in prevs_list[ui]]
            prev_unit = unit_objs[ui - 1] if ui > 0 else None
            for blk in unit_rounds(u, prev_unit, prevs, fillers[ui]):
                blk()
        for blk in drain_unit(unit_objs[-1]):
            blk()

    nc.compile()
    return nc


def _get_nc(apply_qkv_bias, apply_gamma_beta, repeat=1):
    key = (apply_qkv_bias, apply_gamma_beta, repeat)
    if key not in _CACHE:
        _CACHE[key] = _build(*key)
    return _CACHE[key]


def _run(inputs, trace=False):
    from concourse import bass_utils

    a = np.ascontiguousarray(np.asarray(inputs["a"], dtype=np.float32))
    b = np.ascontiguousarray(np.asarray(inputs["b"], dtype=np.float32))
    get = lambda n: np.ascontiguousarray(np.asarray(inputs[n], dtype=np.float32))
    Wq, Wk, Wv = get("Wq"), get("Wk"), get("Wv")
    bq, bk, bv = get("bq"), get("bk"), get("bv")
    gamma, beta = get("gamma"), get("beta")

    apply_qkv_bias = bool(np.any(bq) or np.any(bk) or np.any(bv))
    apply_gamma_beta = bool(np.any(gamma != 1.0) or np.any(beta))
    nc = _get_nc(apply_qkv_bias, apply_gamma_beta)

    in_maps = []
    for c in range(NCORE):
        sl = slice(c * NB, (c + 1) * NB)
        in_maps.append({
            "a": np.ascontiguousarray(a[sl]), "b": np.ascontiguousarray(b[sl]),
            "Wq": Wq, "Wk": Wk, "Wv": Wv,
            "bq": bq, "bk": bk, "bv": bv,
            "gamma": gamma, "beta": beta,
        })
    res = bass_utils.run_bass_kernel_spmd(nc, in_maps,
                                          core_ids=list(range(NCORE)),
                                          trace=trace)
    out = np.concatenate(
        [res.results[c]["out"].reshape(NB, L, C) for c in range(NCORE)], axis=0)
    return out, res


def kernel(**inputs):
    out, _ = _run(inputs, trace=False)
    return out


# revision 20
# speedup vs baseline: 1.0399x; 1.0399x over previous
"""CrossDomainAttention TRN2 kernel: 8-core data-parallel over batch.

Reference computation (per batch element, a/b are (L, C) slices):
  ap = a.T (C, L);  q = ap@Wq.T+bq; k,v from b.T
  attn = softmax(q @ k.T / sqrt(L)) (C, C)
  out = LN(attn @ v + ap) over L, returned as the raw (C*L) buffer viewed (L, C)

v6: fp8 (e4m3) DoubleRow with a j-granular 2-deep software pipeline.
Work is cut into 16 "units" (elem x c-chunk-pair x j); each unit's
rounds interleave on the PE: scores(unit, dp) [4 DR mms + 1 exp on Act]
with half-PV-chains of the previous unit and filler blocks (next
element's DMA/casts/transposes/projections), keeping the PE dense so
HAM stays warm and the drain tail is a single unit's 4 PV chains.
Row-sums are ones-lhsT matmuls at unit end (PSUM in the PV pool),
transposed to per-partition columns via tiny PE transposes.  LN uses
the scale-invariant form (out_pre = rowsum*apT + PV) with a batched
Newton rsqrt.  Residual apT is fp16, transposed from an fp16 copy of a
at 1 cyc/row.
"""

import numpy as np

B, L, C = 16, 512, 2048
NCORE = 8
NB = B // NCORE          # batch elements per core
P = 128
F = 512                  # matmul free-dim tile
NLC = L // P             # 4  l/m chunks
NDB = C // P             # 16 d-blocks / c-blocks
NCCH = C // F            # 4  c chunks
NDP = NDB // 2           # 8  d-pairs (DoubleRow)
NLP = NLC // 2           # 2  l/m pairs (DoubleRow)
LN_EPS = 1e-5
RSTD_SEED = 4.77e-4   # ~1/sqrt(mean var') for the scale-invariant LN form
INV_SQRT_L = 1.0 / float(np.sqrt(L))

_CACHE = {}


def _build(apply_qkv_bias: bool, apply_gamma_beta: bool, repeat: int = 1):
    import concourse.bass as bass
    import concourse.tile as tile
    from concourse import bacc, mybir
    from concourse.bass import ts, ds
    from concourse.masks import make_identity
    from contextlib import ExitStack

    f32 = mybir.dt.float32
    f16 = mybir.dt.float16
    f8 = mybir.dt.float8e4
    AF = mybir.ActivationFunctionType
    ALU = mybir.AluOpType
    DR = mybir.MatmulPerfMode.DoubleRow

    nc = bacc.Bacc("TRN2", target_bir_lowering=False, debug=False,
                   enable_asserts=False)

    a_d = nc.dram_tensor("a", (NB, L, C), f32, kind="ExternalInput").ap()
    b_d = nc.dram_tensor("b", (NB, L, C), f32, kind="ExternalInput").ap()
    w_d = {n: nc.dram_tensor(n, (L, L), f32, kind="ExternalInput").ap()
           for n in ("Wq", "Wk", "Wv")}
    bias_d = {n: nc.dram_tensor(n, (L,), f32, kind="ExternalInput").ap()
              for n in ("bq", "bk", "bv")}
    gamma_d = nc.dram_tensor("gamma", (L,), f32, kind="ExternalInput").ap()
    beta_d = nc.dram_tensor("beta", (L,), f32, kind="ExternalInput").ap()
    out_d = nc.dram_tensor("out", (NB, C, L), f32, kind="ExternalOutput").ap()

    def bcast_p(ap1d):
        return bass.AP(tensor=ap1d.tensor, offset=ap1d.offset,
                       ap=[[0, P]] + [list(d) for d in ap1d.ap])

    ELEMS = [i % NB for i in range(NB * repeat)]

    with tile.TileContext(nc) as tc, ExitStack() as ctx:
        const = ctx.enter_context(tc.tile_pool(name="const", bufs=1))
        ld = ctx.enter_context(tc.tile_pool(name="ld", bufs=2))
        c16 = ctx.enter_context(tc.tile_pool(name="c16", bufs=2))
        epool = ctx.enter_context(tc.tile_pool(name="epool", bufs=2))
        pt = ctx.enter_context(tc.tile_pool(name="pt", bufs=4))
        outp = ctx.enter_context(tc.tile_pool(name="outp", bufs=6))
        small = ctx.enter_context(tc.tile_pool(name="small", bufs=2))
        # PSUM: mm 3x[P,2,F](6 banks) + pv 2x[P,F](2) = 8.  The scores
        # pool is 3 deep so exp (Act) never gates the PE; psr lives in an
        # mm slot, trs in a pv slot.
        ps_mm = ctx.enter_context(tc.tile_pool(name="ps_mm", bufs=3, space="PSUM"))
        ps_pv = ctx.enter_context(tc.tile_pool(name="ps_pv", bufs=2, space="PSUM"))

        def cp(e, dst, src):
            if e is nc.scalar:
                e.copy(dst, src)
            else:
                e.tensor_copy(dst, src)

        # ---- constants ----
        ident16 = const.tile([P, P], f16, tag="ident16")
        make_identity(nc, ident16)
        ident1 = const.tile([1, 1], f32, tag="ident1")
        nc.vector.memset(ident1[:], 1.0)
        ones2 = const.tile([P, 2, 16], f8, tag="ones2")
        nc.vector.memset(ones2[:], 1.0)
        bias_col = {}
        bv_bc = None
        if apply_qkv_bias:
            cpack = const.tile([P, 16], f32, tag="cpack")
            for i, n in enumerate(("bq", "bk")):
                dst = cpack[:, 4 * i: 4 * (i + 1)]
                nc.sync.dma_start(dst, bias_d[n].rearrange("(o p) -> p o", p=P))
                bias_col[n] = dst
            bv_bc = const.tile([P, L], f32, tag="bv_bc")
            nc.sync.dma_start(bv_bc[:], bcast_p(bias_d["bv"]))
        if apply_gamma_beta:
            gb_pack = const.tile([P, 2, L], f32, tag="gb")
            nc.sync.dma_start(gb_pack[:, 0, :], bcast_p(gamma_d))
            nc.sync.dma_start(gb_pack[:, 1, :], bcast_p(beta_d))

        WT = {n: const.tile([P, NLC, L], f8, tag=f"WT_{n}", name=f"WT_{n}")
              for n in ("Wq", "Wk", "Wv")}

        # ---------- emission helpers ----------
        w_eng = [nc.vector, nc.scalar]

        def load_weight_blocks(n, wi):
            def emit():
                wld = ld.tile([P, NLC, F], f32, tag="ld", name=f"wld_{n}")
                nc.sync.dma_start(wld[:],
                                  w_d[n].rearrange("(o p) l -> p o l", p=P))
                w16 = c16.tile([P, NLC, F], f16, tag="w16", bufs=1,
                               name=f"w16_{n}")
                nc.vector.tensor_copy(w16[:], wld[:])
                for li in range(NLC):
                    pst = ps_mm.tile([P, F], f16, tag="mm", name="wtp")
                    for mo in range(NLC):
                        nc.tensor.transpose(pst[:, ts(mo, P)],
                                            w16[:, mo, ts(li, P)], ident16[:])
                    cp(w_eng[(wi + li) % 2], WT[n][:, li, :], pst[:])
            return [emit]

        class E:
            pass

        def make_elem(bi, rep):
            e = E()
            e.bi = bi
            e.rep = rep
            e.a8 = epool.tile([P, NLC, C], f8, tag="a8", name=f"a8_{rep}")
            e.b8 = epool.tile([P, NLC, C], f8, tag="b8", name=f"b8_{rep}")
            e.qT = epool.tile([P, NLC, C], f8, tag="qT", name=f"qT_{rep}")
            e.kT = epool.tile([P, NLC, C], f8, tag="kT", name=f"kT_{rep}")
            e.v8 = epool.tile([P, NDB, L], f8, tag="v8", name=f"v8_{rep}")
            e.apT = epool.tile([P, NDB, L], f16, tag="apT", name=f"apT_{rep}")
            e.a16 = {}
            return e

        a_cast8 = [nc.vector, nc.scalar, nc.vector, nc.scalar]
        b_cast8 = [nc.scalar, nc.vector, nc.scalar, nc.vector]
        apt_cp = [nc.vector, nc.scalar, nc.vector, nc.scalar]

        def load_a_chunk(e, li):
            def emit():
                ach = ld.tile([P, C], f32, tag="ld", name=f"a_{e.rep}_{li}")
                nc.sync.dma_start(ach[:], a_d[e.bi, ds(li * P, P), :])
                a16 = c16.tile([P, C], f16, tag="a16", bufs=4,
                               name=f"a16_{e.rep}_{li}")
                nc.vector.tensor_copy(a16[:], ach[:])
                cp(a_cast8[li], e.a8[:, li, :], ach[:])
                e.a16[li] = a16
            return [emit]

        def apt_blocks(e, li):
            # 2 transposes per sub-block; copy after the 2nd sub-block
            state = {}

            def mk(g, h):
                def emit():
                    a16 = e.a16[li]
                    if h == 0:
                        state[g] = ps_mm.tile([P, F], f16, tag="mm",
                                              name="atp")
                    pst = state[g]
                    for j in range(2):
                        db = 4 * g + 2 * h + j
                        nc.tensor.transpose(pst[:, ts(2 * h + j, P)],
                                            a16[:, ts(db, P)], ident16[:])
                    if h == 1:
                        dst = e.apT[:, 4 * g:4 * g + 4, ts(li, P)]
                        cp(apt_cp[(li + g) % 2],
                           dst, pst[:].rearrange("p (b f) -> p b f", f=P))
                return emit
            return [mk(g, h) for g in range(NLC) for h in range(2)]

        def load_b_split(e):
            tiles = {}

            def dma_blk():
                for li in range(NLC):
                    bch = ld.tile([P, C], f32, tag="ld",
                                  name=f"b_{e.rep}_{li}")
                    nc.sync.dma_start(bch[:], b_d[e.bi, ds(li * P, P), :])
                    tiles[li] = bch

            def mk_cast(li):
                def emit():
                    cp(b_cast8[li], e.b8[:, li, :], tiles[li])
                return emit
            return [dma_blk], [mk_cast(li) for li in range(NLC)]

        qk_cp = [nc.vector, nc.scalar]

        def qk_proj_blocks(e, wname, bname, src_name, dst_name):
            def mk(mi):
                def emit():
                    src = getattr(e, src_name)
                    dst = getattr(e, dst_name)
                    pss = [ps_mm.tile([P, F], f32, tag="mm", name=f"qk{i}")
                           for i in range(4)]
                    for lp in range(NLP):
                        for ci in range(NCCH):
                            nc.tensor.matmul(
                                pss[ci][:],
                                lhsT=WT[wname][:, 2 * lp:2 * lp + 2, ts(mi, P)],
                                rhs=src[:, 2 * lp:2 * lp + 2, ts(ci, F)],
                                start=(lp == 0), stop=(lp == NLP - 1),
                                perf_mode=DR)
                    for ci in range(NCCH):
                        dslice = dst[:, mi, ts(ci, F)]
                        if apply_qkv_bias:
                            nc.scalar.activation(
                                dslice, pss[ci][:], AF.Identity,
                                bias=bias_col[bname][:, mi:mi + 1])
                        else:
                            cp(qk_cp[ci % 2], dslice, pss[ci][:])
                return emit
            return [mk(mi) for mi in range(NLC)]

        def v_proj_blocks(e):
            def mk(dp):
                def emit():
                    pss = [ps_mm.tile([P, F], f32, tag="mm", name=f"v{i}")
                           for i in range(2)]
                    for s in range(2):
                        di = 2 * dp + s
                        for lp in range(NLP):
                            nc.tensor.matmul(
                                pss[s][:],
                                lhsT=e.b8[:, 2 * lp:2 * lp + 2, ts(di, P)],
                                rhs=WT["Wv"][:, 2 * lp:2 * lp + 2, :],
                                start=(lp == 0), stop=(lp == NLP - 1),
                                perf_mode=DR)
                    for s in range(2):
                        cp(qk_cp[(dp + s) % 2], e.v8[:, 2 * dp + s, :],
                           pss[s][:])
                        if apply_qkv_bias:
                            nc.vector.tensor_add(e.v8[:, 2 * dp + s, :],
                                                 e.v8[:, 2 * dp + s, :],
                                                 bv_bc[:, :])
                return emit
            return [mk(dp) for dp in range(NDP)]

        # ---------- attention units ----------

        class Unit:
            # one (elem, p, j): a 1024-row slab of the attention output
            def __init__(u, e, p, j):
                u.e, u.p, u.j = e, p, j
                u.PT = pt.tile([P, NDB, F], f8, tag="pt",
                               name=f"pt_{e.rep}_{p}_{j}")
                u.psr = None
                u.rs_post = None
                u.rs_cols = None
                u.stats = small.tile([P, NCCH, 2], f32, tag="stats",
                                     name=f"st_{e.rep}_{p}_{j}")
                u.chain = {}
                u.outs = {}

        def scores_round(u, dp):
            # 4 DR mms -> [P,2,F] psum; 1 exp (Act)
            def emit():
                e = u.e
                pss = ps_mm.tile([P, 2, F], f32, tag="mm", name="sc")
                for sb in range(2):
                    di = 2 * dp + sb
                    for mp in range(NLP):
                        nc.tensor.matmul(
                            pss[:, sb, :],
                            lhsT=e.kT[:, 2 * mp:2 * mp + 2, ts(di, P)],
                            rhs=e.qT[:, 2 * mp:2 * mp + 2,
                                     ts(2 * u.p + u.j, F)],
                            start=(mp == 0), stop=(mp == NLP - 1),
                            perf_mode=DR)
                nc.scalar.activation(u.PT[:, 2 * dp:2 * dp + 2, :],
                                     pss[:], AF.Exp, scale=INV_SQRT_L)
            return emit

        def rs_block(u):
            # 8 N=512 ones-lhsT mms -> psr row; psrow copy (Act); 4 tiny
            # transposes; copy columns to SBUF.
            def emit_mms():
                u.psr = ps_mm.tile([16, F], f32, tag="mm", name="psr")
                for dp in range(NDP):
                    nc.tensor.matmul(u.psr[:],
                                     lhsT=ones2[:],
                                     rhs=u.PT[:, 2 * dp:2 * dp + 2, :],
                                     start=(dp == 0), stop=(dp == NDP - 1),
                                     perf_mode=DR,
                                     skip_group_check=True)

            def emit_post():
                psrow = small.tile([1, F], f32, tag="rrow", name="psrow")
                nc.scalar.copy(psrow[:], u.psr[0:1, :])
                trs_ps = ps_pv.tile([P, NCCH], f32, tag="pv", name="trs")
                for cb in range(NCCH):
                    nc.tensor.transpose(trs_ps[:, cb:cb + 1],
                                        psrow[0:1, ts(cb, P)],
                                        ident1[:])
                u.rs_cols = small.tile([P, NCCH], f32, tag="rcol",
                                       name="rs_cols")
                nc.vector.tensor_copy(u.rs_cols[:], trs_ps[:])
            return [emit_mms, emit_post]

        def pv_half(u, cb, h):
            # half of one PV chain: 4 DR mms; on h==1 also stt+bn (DVE)
            def emit():
                e = u.e
                if h == 0:
                    u.chain[cb] = ps_pv.tile([P, L], f32, tag="pv",
                                             name="po")
                po = u.chain[cb]
                for dp in range(4 * h, 4 * h + 4):
                    nc.tensor.matmul(
                        po[:],
                        lhsT=u.PT[:, 2 * dp:2 * dp + 2, ts(cb, P)],
                        rhs=e.v8[:, 2 * dp:2 * dp + 2, :],
                        start=(dp == 0), stop=(dp == NDP - 1),
                        perf_mode=DR)
                if h == 1:
                    gb = (2 * u.p + u.j) * NCCH + cb
                    out_sb = outp.tile([P, L], f16, tag="out",
                                       name="out_sb")
                    nc.vector.scalar_tensor_tensor(
                        out_sb[:], e.apT[:, gb, :],
                        u.rs_cols[:, cb:cb + 1], po[:], ALU.mult, ALU.add)
                    st6 = small.tile([P, 6], f32, tag="st6", name="st6")
                    nc.vector.bn_stats(st6[:], out_sb[:])
                    nc.vector.bn_aggr(u.stats[:, cb, :], st6[:])
                    u.outs[cb] = out_sb
            return emit

        def ln_finish(u, cb0=0, ncb=NCCH):
            # batched Newton rsqrt over ncb cb's; final scale + DMA
            def emit():
                e = u.e
                var_ap = u.stats[:, cb0:cb0 + ncb, 1]
                y = small.tile([P, 2, NCCH], f32, tag="nwt", name="nwt")
                y = y[:, :, :ncb]
                nc.vector.tensor_scalar(y[:, 0, :], var_ap,
                                        -0.5 * RSTD_SEED ** 3,
                                        1.5 * RSTD_SEED,
                                        ALU.mult, ALU.add)
                for _ in range(3):
                    t = y[:, 1, :]
                    nc.vector.tensor_mul(t, y[:, 0, :], y[:, 0, :])
                    nc.vector.tensor_mul(t, t, var_ap)
                    nc.vector.tensor_scalar(t, t, -0.5, 1.5,
                                            ALU.mult, ALU.add)
                    nc.vector.tensor_mul(y[:, 0, :], y[:, 0, :], t)
                for i, cb in enumerate(range(cb0, cb0 + ncb)):
                    gb = (2 * u.p + u.j) * NCCH + cb
                    out_sb = u.outs[cb]
                    out32 = outp.tile([P, L], f32, tag="out32",
                                      bufs=3, name="out32")
                    nc.vector.tensor_scalar(out32[:], out_sb[:],
                                            u.stats[:, cb, 0:1],
                                            y[:, 0, i:i + 1],
                                            ALU.subtract, ALU.mult)
                    if apply_gamma_beta:
                        nc.vector.tensor_mul(out32[:], out32[:],
                                             gb_pack[:, 0, :])
                        nc.vector.tensor_add(out32[:], out32[:],
                                             gb_pack[:, 1, :])
                    nc.sync.dma_start(out_d[e.bi, ds(gb * P, P), :],
                                      out32[:])
            return emit

        def unit_rounds(u, prev_unit, prevs, fillers):
            # 8 rounds of: scores(dp) + prev half-chains + fillers
            halves = []
            for pu in prevs:
                for cb in range(NCCH):
                    halves.append(pv_half(pu, cb, 0))
                    halves.append(pv_half(pu, cb, 1))
            per_round = (len(halves) + NDP - 1) // NDP if halves else 0
            blocks = []
            hi = 0
            for dp in range(NDP):
                blocks.append(scores_round(u, dp))
                for _ in range(per_round):
                    if hi < len(halves):
                        blocks.append(halves[hi])
                        hi += 1
                if dp == 0 and prev_unit is not None:
                    blocks.append(prev_unit.rs_post)
                if fillers:
                    blocks.append(fillers.pop(0))
                    if len(fillers) > 2 * (NDP - 1 - dp):
                        blocks.append(fillers.pop(0))
            while hi < len(halves):
                blocks.append(halves[hi])
                hi += 1
            rsb = rs_block(u)
            blocks.append(rsb[0])
            u.rs_post = rsb[1]
            for pu in prevs:
                blocks.append(ln_finish(pu))
            while fillers:
                blocks.append(fillers.pop(0))
            return blocks

        def drain_unit(u):
            blocks = [u.rs_post]
            for cb in range(NCCH):
                blocks.append(pv_half(u, cb, 0))
                blocks.append(pv_half(u, cb, 1))
                if cb % 2 == 1:
                    blocks.append(ln_finish(u, cb - 1, 2))
            return blocks

        # ---------- schedule ----------
        elems = [make_elem(bi, rep) for rep, bi in enumerate(ELEMS)]

        def load_a_blocks(e):
            return [blk for li in range(NLC) for blk in load_a_chunk(e, li)]

        def elem_apt_blocks(e):
            return [blk for li in range(NLC) for blk in apt_blocks(e, li)]

        # head: critical path Wq+Wk+a0+b0 DMA -> q,k proj -> scores.
        e0 = elems[0]
        la = load_a_blocks(e0)
        b_dma0, b_casts0 = load_b_split(e0)
        head = []
        head += load_weight_blocks("Wq", 0)
        head += [la[0], la[1]]
        head += load_weight_blocks("Wk", 1)
        head += [la[2], la[3]]
        head += b_dma0
        head += qk_proj_blocks(e0, "Wq", "bq", "a8", "qT")
        head += elem_apt_blocks(e0)
        head += b_casts0
        head += qk_proj_blocks(e0, "Wk", "bk", "b8", "kT")
        for blk in head:
            blk()

        # unit pipeline.  fillers[ui] rides unit ui's rounds.
        units = []
        for e in elems:
            for p in range(2):
                for j in range(2):
                    units.append((e, p, j))
        NU = len(units)
        fillers = [[] for _ in range(NU)]
        # elem0's v-proj (Wv lands after b0) rides unit 1; unit 0's PV
        # is deferred one unit.
        fillers[1] = load_weight_blocks("Wv", 0) + v_proj_blocks(e0)
        for ei in range(len(elems) - 1):
            en = elems[ei + 1]
            u0 = 4 * ei
            apt = elem_apt_blocks(en)
            b_dma, b_casts = load_b_split(en)
            qb = qk_proj_blocks(en, "Wq", "bq", "a8", "qT")
            kb = qk_proj_blocks(en, "Wk", "bk", "b8", "kT")
            fillers[u0 + 0] += load_a_blocks(en) + apt[:8]
            fillers[u0 + 1] += b_dma + apt[8:24]
            fillers[u0 + 2] += apt[24:] + qb[:2]
            fillers[u0 + 3] += qb[2:] + b_casts + kb
            fillers[u0 + 4] += v_proj_blocks(en)

        prevs_list = [[] for _ in range(NU)]
        for ui in range(2, NU):
            prevs_list[ui] = [0, 1] if ui == 2 else [ui - 1]

        unit_objs = []
        for ui, (e, p, j) in enumerate(units):
            u = Unit(e, p, j)
            unit_objs.append(u)
            prevs = [unit_objs[k] for k in prevs_list[ui]]
            prev_unit = unit_objs[ui - 1] if ui > 0 else None
            for blk in unit_rounds(u, prev_unit, prevs, fillers[ui]):
                blk()
        for blk in drain_unit(unit_objs[-1]):
            blk()

    nc.compile()
    return nc


def _get_nc(apply_qkv_bias, apply_gamma_beta, repeat=1):
    key = (apply_qkv_bias, apply_gamma_beta, repeat)
    if key not in _CACHE:
        _CACHE[key] = _build(*key)
    return _CACHE[key]


def _run(inputs, trace=False):
    from concourse import bass_utils

    a = np.ascontiguousarray(np.asarray(inputs["a"], dtype=np.float32))
    b = np.ascontiguousarray(np.asarray(inputs["b"], dtype=np.float32))
    get = lambda n: np.ascontiguousarray(np.asarray(inputs[n], dtype=np.float32))
    Wq, Wk, Wv = get("Wq"), get("Wk"), get("Wv")
    bq, bk, bv = get("bq"), get("bk"), get("bv")
    gamma, beta = get("gamma"), get("beta")

    apply_qkv_bias = bool(np.any(bq) or np.any(bk) or np.any(bv))
    apply_gamma_beta = bool(np.any(gamma != 1.0) or np.any(beta))
    nc = _get_nc(apply_qkv_bias, apply_gamma_beta)

    in_maps = []
    for c in range(NCORE):
        sl = slice(c * NB, (c + 1) * NB)
        in_maps.append({
            "a": np.ascontiguousarray(a[sl]), "b": np.ascontiguousarray(b[sl]),
            "Wq": Wq, "Wk": Wk, "Wv": Wv,
            "bq": bq, "bk": bk, "bv": bv,
            "gamma": gamma, "beta": beta,
        })
    res = bass_utils.run_bass_kernel_spmd(nc, in_maps,
                                          core_ids=list(range(NCORE)),
                                          trace=trace)
    out = np.concatenate(
        [res.results[c]["out"].reshape(NB, L, C) for c in range(NCORE)], axis=0)
    return out, res


def kernel(**inputs):
    out, _ = _run(inputs, trace=False)
    return out


# revision 22
# speedup vs baseline: 1.0739x; 1.0327x over previous
"""CrossDomainAttention TRN2 kernel: 8-core data-parallel over batch.

Reference computation (per batch element, a/b are (L, C) slices):
  ap = a.T (C, L);  q = ap@Wq.T+bq; k,v from b.T
  attn = softmax(q @ k.T / sqrt(L)) (C, C)
  out = LN(attn @ v + ap) over L, returned as the raw (C*L) buffer viewed (L, C)

v6: fp8 (e4m3) DoubleRow with a j-granular 2-deep software pipeline.
Work is cut into 16 "units" (elem x c-chunk-pair x j); each unit's
rounds interleave on the PE: scores(unit, dp) [4 DR mms + 1 exp on Act]
with half-PV-chains of the previous unit and filler blocks (next
element's DMA/casts/transposes/projections), keeping the PE dense so
HAM stays warm and the drain tail is a single unit's 4 PV chains.
Row-sums are ones-lhsT matmuls at unit end (PSUM in the PV pool),
transposed to per-partition columns via tiny PE transposes.  LN uses
the scale-invariant form (out_pre = rowsum*apT + PV) with a batched
Newton rsqrt.  Residual apT is fp16, transposed from an fp16 copy of a
at 1 cyc/row.
"""

import numpy as np

B, L, C = 16, 512, 2048
NCORE = 8
NB = B // NCORE          # batch elements per core
P = 128
F = 512                  # matmul free-dim tile
NLC = L // P             # 4  l/m chunks
NDB = C // P             # 16 d-blocks / c-blocks
NCCH = C // F            # 4  c chunks
NDP = NDB // 2           # 8  d-pairs (DoubleRow)
NLP = NLC // 2           # 2  l/m pairs (DoubleRow)
LN_EPS = 1e-5
RSTD_SEED = 4.77e-4   # ~1/sqrt(mean var') for the scale-invariant LN form
INV_SQRT_L = 1.0 / float(np.sqrt(L))
MSCALE = 64.0   # M = Wq^T Wk is stored as M*MSCALE so fp8e4 stays normal

_CACHE = {}


def _build(apply_qkv_bias: bool, apply_gamma_beta: bool, repeat: int = 1):
    import concourse.bass as bass
    import concourse.tile as tile
    from concourse import bacc, mybir
    from concourse.bass import ts, ds
    from concourse.masks import make_identity
    from contextlib import ExitStack

    f32 = mybir.dt.float32
    f16 = mybir.dt.float16
    f8 = mybir.dt.float8e4
    AF = mybir.ActivationFunctionType
    ALU = mybir.AluOpType
    DR = mybir.MatmulPerfMode.DoubleRow

    nc = bacc.Bacc("TRN2", target_bir_lowering=False, debug=False,
                   enable_asserts=False)

    a_d = nc.dram_tensor("a", (NB, L, C), f32, kind="ExternalInput").ap()
    b_d = nc.dram_tensor("b", (NB, L, C), f32, kind="ExternalInput").ap()
    w_d = {n: nc.dram_tensor(n, (L, L), f32, kind="ExternalInput").ap()
           for n in ("Wq", "Wk", "Wv")}
    bias_d = {n: nc.dram_tensor(n, (L,), f32, kind="ExternalInput").ap()
              for n in ("bq", "bk", "bv")}
    gamma_d = nc.dram_tensor("gamma", (L,), f32, kind="ExternalInput").ap()
    beta_d = nc.dram_tensor("beta", (L,), f32, kind="ExternalInput").ap()
    out_d = nc.dram_tensor("out", (NB, C, L), f32, kind="ExternalOutput").ap()

    def bcast_p(ap1d):
        return bass.AP(tensor=ap1d.tensor, offset=ap1d.offset,
                       ap=[[0, P]] + [list(d) for d in ap1d.ap])

    ELEMS = [i % NB for i in range(NB * repeat)]

    with tile.TileContext(nc) as tc, ExitStack() as ctx:
        const = ctx.enter_context(tc.tile_pool(name="const", bufs=1))
        ld = ctx.enter_context(tc.tile_pool(name="ld", bufs=2))
        c16 = ctx.enter_context(tc.tile_pool(name="c16", bufs=2))
        epool = ctx.enter_context(tc.tile_pool(name="epool", bufs=2))
        pt = ctx.enter_context(tc.tile_pool(name="pt", bufs=4))
        outp = ctx.enter_context(tc.tile_pool(name="outp", bufs=6))
        small = ctx.enter_context(tc.tile_pool(name="small", bufs=2))
        # PSUM: mm 3x[P,2,F](6 banks) + pv 2x[P,F](2) = 8.  The scores
        # pool is 3 deep so exp (Act) never gates the PE; psr lives in an
        # mm slot, trs in a pv slot.
        ps_mm = ctx.enter_context(tc.tile_pool(name="ps_mm", bufs=3, space="PSUM"))
        ps_pv = ctx.enter_context(tc.tile_pool(name="ps_pv", bufs=2, space="PSUM"))

        def cp(e, dst, src):
            if e is nc.scalar:
                e.copy(dst, src)
            else:
                e.tensor_copy(dst, src)

        # ---- constants ----
        ident16 = const.tile([P, P], f16, tag="ident16")
        make_identity(nc, ident16)
        ident1 = const.tile([1, 1], f32, tag="ident1")
        nc.vector.memset(ident1[:], 1.0)
        ones2 = const.tile([P, 2, 16], f8, tag="ones2")
        nc.vector.memset(ones2[:], 1.0)
        bias_col = {}
        bv_bc = None
        if apply_qkv_bias:
            cpack = const.tile([P, 16], f32, tag="cpack")
            for i, n in enumerate(("bq", "bk")):
                dst = cpack[:, 4 * i: 4 * (i + 1)]
                nc.sync.dma_start(dst, bias_d[n].rearrange("(o p) -> p o", p=P))
                bias_col[n] = dst
            bv_bc = const.tile([P, L], f32, tag="bv_bc")
            nc.sync.dma_start(bv_bc[:], bcast_p(bias_d["bv"]))
        if apply_gamma_beta:
            gb_pack = const.tile([P, 2, L], f32, tag="gb")
            nc.sync.dma_start(gb_pack[:, 0, :], bcast_p(gamma_d))
            nc.sync.dma_start(gb_pack[:, 1, :], bcast_p(beta_d))

        wt_names = ("Wq", "Wk", "Wv") if apply_qkv_bias else ("Wv",)
        WT = {n: const.tile([P, NLC, L], f8, tag=f"WT_{n}", name=f"WT_{n}")
              for n in wt_names}
        if not apply_qkv_bias:
            M8 = const.tile([P, NLC, L], f8, tag="M8", name="M8")

        # ---------- emission helpers ----------
        w_eng = [nc.vector, nc.scalar]

        def load_weight_blocks(n, wi):
            def emit():
                wld = ld.tile([P, NLC, F], f32, tag="ld", name=f"wld_{n}")
                nc.sync.dma_start(wld[:],
                                  w_d[n].rearrange("(o p) l -> p o l", p=P))
                w16 = c16.tile([P, NLC, F], f16, tag="w16", bufs=2,
                               name=f"w16_{n}")
                nc.vector.tensor_copy(w16[:], wld[:])
                for li in range(NLC):
                    pst = ps_mm.tile([P, F], f16, tag="mm", name="wtp")
                    for mo in range(NLC):
                        nc.tensor.transpose(pst[:, ts(mo, P)],
                                            w16[:, mo, ts(li, P)], ident16[:])
                    cp(w_eng[(wi + li) % 2], WT[n][:, li, :], pst[:])
            return [emit]

        def load_m_blocks():
            # scores = (a^T (Wq^T Wk)) b: precompute M8 = Wq^T Wk * MSCALE
            # directly in the [l_p, li, l'] weight layout (no transposes).
            w16s = {}

            def ld_one(n):
                def emit():
                    wld = ld.tile([P, NLC, F], f32, tag="ld",
                                  name=f"wld_{n}")
                    nc.sync.dma_start(
                        wld[:], w_d[n].rearrange("(o p) l -> p o l", p=P))
                    w16 = c16.tile([P, NLC, F], f16, tag="w16", bufs=2,
                                   name=f"w16_{n}")
                    nc.vector.tensor_copy(w16[:], wld[:])
                    w16s[n] = w16
                return emit

            def mk_mm(li):
                def emit():
                    ps = ps_mm.tile([P, F], f32, tag="mm", name="mps")
                    for mo in range(NLC):
                        nc.tensor.matmul(
                            ps[:],
                            lhsT=w16s["Wq"][:, mo, ts(li, P)],
                            rhs=w16s["Wk"][:, mo, :],
                            start=(mo == 0), stop=(mo == NLC - 1))
                    nc.scalar.activation(M8[:, li, :], ps[:], AF.Copy,
                                         scale=MSCALE)
                return emit
            return [ld_one("Wq"), ld_one("Wk")] + [mk_mm(li)
                                                   for li in range(NLC)]

        class E:
            pass

        def make_elem(bi, rep):
            e = E()
            e.bi = bi
            e.rep = rep
            e.a8 = epool.tile([P, NLC, C], f8, tag="a8", name=f"a8_{rep}")
            e.b8 = epool.tile([P, NLC, C], f8, tag="b8", name=f"b8_{rep}")
            e.qT = epool.tile([P, NLC, C], f8, tag="qT", name=f"qT_{rep}")
            e.kT = epool.tile([P, NLC, C], f8, tag="kT", name=f"kT_{rep}")
            e.v8 = epool.tile([P, NDB, L], f8, tag="v8", name=f"v8_{rep}")
            e.apT = epool.tile([P, NDB, L], f16, tag="apT", name=f"apT_{rep}")
            e.a16 = {}
            return e

        a_cast8 = [nc.vector, nc.scalar, nc.vector, nc.scalar]
        b_cast8 = [nc.scalar, nc.vector, nc.scalar, nc.vector]
        apt_cp = [nc.vector, nc.scalar, nc.vector, nc.scalar]

        def load_a_chunk(e, li):
            def emit():
                ach = ld.tile([P, C], f32, tag="ld", name=f"a_{e.rep}_{li}")
                nc.sync.dma_start(ach[:], a_d[e.bi, ds(li * P, P), :])
                a16 = c16.tile([P, C], f16, tag="a16", bufs=4,
                               name=f"a16_{e.rep}_{li}")
                nc.vector.tensor_copy(a16[:], ach[:])
                cp(a_cast8[li], e.a8[:, li, :], ach[:])
                e.a16[li] = a16
            return [emit]

        def apt_blocks(e, li):
            # 2 transposes per sub-block; copy after the 2nd sub-block
            state = {}

            def mk(g, h):
                def emit():
                    a16 = e.a16[li]
                    if h == 0:
                        state[g] = ps_mm.tile([P, F], f16, tag="mm",
                                              name="atp")
                    pst = state[g]
                    for j in range(2):
                        db = 4 * g + 2 * h + j
                        nc.tensor.transpose(pst[:, ts(2 * h + j, P)],
                                            a16[:, ts(db, P)], ident16[:])
                    if h == 1:
                        dst = e.apT[:, 4 * g:4 * g + 4, ts(li, P)]
                        cp(apt_cp[(li + g) % 2],
                           dst, pst[:].rearrange("p (b f) -> p b f", f=P))
                return emit
            return [mk(g, h) for g in range(NLC) for h in range(2)]

        def load_b_split(e):
            tiles = {}

            def dma_blk():
                for li in range(NLC):
                    bch = ld.tile([P, C], f32, tag="ld",
                                  name=f"b_{e.rep}_{li}")
                    nc.sync.dma_start(bch[:], b_d[e.bi, ds(li * P, P), :])
                    tiles[li] = bch

            def mk_cast(li):
                def emit():
                    cp(b_cast8[li], e.b8[:, li, :], tiles[li])
                return emit
            return [dma_blk], [mk_cast(li) for li in range(NLC)]

        qk_cp = [nc.vector, nc.scalar]

        def qk_proj_blocks(e, wname, bname, src_name, dst_name):
            def mk(mi):
                def emit():
                    wt = (M8 if (wname == "Wq" and not apply_qkv_bias)
                          else WT[wname])
                    src = getattr(e, src_name)
                    dst = getattr(e, dst_name)
                    pss = [ps_mm.tile([P, F], f32, tag="mm", name=f"qk{i}")
                           for i in range(4)]
                    for lp in range(NLP):
                        for ci in range(NCCH):
                            nc.tensor.matmul(
                                pss[ci][:],
                                lhsT=wt[:, 2 * lp:2 * lp + 2, ts(mi, P)],
                                rhs=src[:, 2 * lp:2 * lp + 2, ts(ci, F)],
                                start=(lp == 0), stop=(lp == NLP - 1),
                                perf_mode=DR)
                    for ci in range(NCCH):
                        dslice = dst[:, mi, ts(ci, F)]
                        if apply_qkv_bias:
                            nc.scalar.activation(
                                dslice, pss[ci][:], AF.Identity,
                                bias=bias_col[bname][:, mi:mi + 1])
                        else:
                            cp(qk_cp[ci % 2], dslice, pss[ci][:])
                return emit
            return [mk(mi) for mi in range(NLC)]

        def v_proj_blocks(e):
            def mk(dp):
                def emit():
                    pss = [ps_mm.tile([P, F], f32, tag="mm", name=f"v{i}")
                           for i in range(2)]
                    for s in range(2):
                        di = 2 * dp + s
                        for lp in range(NLP):
                            nc.tensor.matmul(
                                pss[s][:],
                                lhsT=e.b8[:, 2 * lp:2 * lp + 2, ts(di, P)],
                                rhs=WT["Wv"][:, 2 * lp:2 * lp + 2, :],
                                start=(lp == 0), stop=(lp == NLP - 1),
                                perf_mode=DR)
                    for s in range(2):
                        cp(qk_cp[(dp + s) % 2], e.v8[:, 2 * dp + s, :],
                           pss[s][:])
                        if apply_qkv_bias:
                            nc.vector.tensor_add(e.v8[:, 2 * dp + s, :],
                                                 e.v8[:, 2 * dp + s, :],
                                                 bv_bc[:, :])
                return emit
            return [mk(dp) for dp in range(NDP)]

        # ---------- attention units ----------

        class Unit:
            # one (elem, p, j): a 1024-row slab of the attention output
            def __init__(u, e, p, j):
                u.e, u.p, u.j = e, p, j
                u.PT = pt.tile([P, NDB, F], f8, tag="pt",
                               name=f"pt_{e.rep}_{p}_{j}")
                u.psr = None
                u.rs_post = None
                u.rs_cols = None
                u.stats = small.tile([P, NCCH, 2], f32, tag="stats",
                                     name=f"st_{e.rep}_{p}_{j}")
                u.chain = {}
                u.outs = {}

        def scores_round(u, dp):
            # 4 DR mms -> [P,2,F] psum; 1 exp (Act)
            def emit():
                e = u.e
                lh = e.kT if apply_qkv_bias else e.b8
                esc = INV_SQRT_L if apply_qkv_bias else INV_SQRT_L / MSCALE
                pss = ps_mm.tile([P, 2, F], f32, tag="mm", name="sc")
                for sb in range(2):
                    di = 2 * dp + sb
                    for mp in range(NLP):
                        nc.tensor.matmul(
                            pss[:, sb, :],
                            lhsT=lh[:, 2 * mp:2 * mp + 2, ts(di, P)],
                            rhs=e.qT[:, 2 * mp:2 * mp + 2,
                                     ts(2 * u.p + u.j, F)],
                            start=(mp == 0), stop=(mp == NLP - 1),
                            perf_mode=DR)
                nc.scalar.activation(u.PT[:, 2 * dp:2 * dp + 2, :],
                                     pss[:], AF.Exp, scale=esc)
            return emit

        def rs_block(u):
            # 8 N=512 ones-lhsT mms -> psr row; psrow copy (Act); 4 tiny
            # transposes; copy columns to SBUF.
            def emit_mms():
                u.psr = ps_mm.tile([16, F], f32, tag="mm", name="psr")
                for dp in range(NDP):
                    nc.tensor.matmul(u.psr[:],
                                     lhsT=ones2[:],
                                     rhs=u.PT[:, 2 * dp:2 * dp + 2, :],
                                     start=(dp == 0), stop=(dp == NDP - 1),
                                     perf_mode=DR,
                                     skip_group_check=True)

            def emit_post():
                psrow = small.tile([1, F], f32, tag="rrow", name="psrow")
                nc.scalar.copy(psrow[:], u.psr[0:1, :])
                trs_ps = ps_pv.tile([P, NCCH], f32, tag="pv", name="trs")
                for cb in range(NCCH):
                    nc.tensor.transpose(trs_ps[:, cb:cb + 1],
                                        psrow[0:1, ts(cb, P)],
                                        ident1[:])
                u.rs_cols = small.tile([P, NCCH], f32, tag="rcol",
                                       name="rs_cols")
                nc.vector.tensor_copy(u.rs_cols[:], trs_ps[:])
            return [emit_mms, emit_post]

        def pv_half(u, cb, h):
            # half of one PV chain: 4 DR mms; on h==1 also stt+bn (DVE)
            def emit():
                e = u.e
                if h == 0:
                    u.chain[cb] = ps_pv.tile([P, L], f32, tag="pv",
                                             name="po")
                po = u.chain[cb]
                for dp in range(4 * h, 4 * h + 4):
                    nc.tensor.matmul(
                        po[:],
                        lhsT=u.PT[:, 2 * dp:2 * dp + 2, ts(cb, P)],
                        rhs=e.v8[:, 2 * dp:2 * dp + 2, :],
                        start=(dp == 0), stop=(dp == NDP - 1),
                        perf_mode=DR)
                if h == 1:
                    gb = (2 * u.p + u.j) * NCCH + cb
                    out_sb = outp.tile([P, L], f16, tag="out",
                                       name="out_sb")
                    nc.vector.scalar_tensor_tensor(
                        out_sb[:], e.apT[:, gb, :],
                        u.rs_cols[:, cb:cb + 1], po[:], ALU.mult, ALU.add)
                    st6 = small.tile([P, 6], f32, tag="st6", name="st6")
                    nc.vector.bn_stats(st6[:], out_sb[:])
                    nc.vector.bn_aggr(u.stats[:, cb, :], st6[:])
                    u.outs[cb] = out_sb
            return emit

        def ln_finish(u, cb0=0, ncb=NCCH):
            # batched Newton rsqrt over ncb cb's; final scale + DMA
            def emit():
                e = u.e
                var_ap = u.stats[:, cb0:cb0 + ncb, 1]
                y = small.tile([P, 2, NCCH], f32, tag="nwt", name="nwt")
                y = y[:, :, :ncb]
                nc.vector.tensor_scalar(y[:, 0, :], var_ap,
                                        -0.5 * RSTD_SEED ** 3,
                                        1.5 * RSTD_SEED,
                                        ALU.mult, ALU.add)
                for _ in range(3):
                    t = y[:, 1, :]
                    nc.vector.tensor_mul(t, y[:, 0, :], y[:, 0, :])
                    nc.vector.tensor_mul(t, t, var_ap)
                    nc.vector.tensor_scalar(t, t, -0.5, 1.5,
                                            ALU.mult, ALU.add)
                    nc.vector.tensor_mul(y[:, 0, :], y[:, 0, :], t)
                for i, cb in enumerate(range(cb0, cb0 + ncb)):
                    gb = (2 * u.p + u.j) * NCCH + cb
                    out_sb = u.outs[cb]
                    out32 = outp.tile([P, L], f32, tag="out32",
                                      bufs=3, name="out32")
                    nc.vector.tensor_scalar(out32[:], out_sb[:],
                                            u.stats[:, cb, 0:1],
                                            y[:, 0, i:i + 1],
                                            ALU.subtract, ALU.mult)
                    if apply_gamma_beta:
                        nc.vector.tensor_mul(out32[:], out32[:],
                                             gb_pack[:, 0, :])
                        nc.vector.tensor_add(out32[:], out32[:],
                                             gb_pack[:, 1, :])
                    nc.sync.dma_start(out_d[e.bi, ds(gb * P, P), :],
                                      out32[:])
            return emit

        def unit_rounds(u, prev_unit, prevs, fillers):
            # 8 rounds of: scores(dp) + prev half-chains + fillers
            halves = []
            for pu in prevs:
                for cb in range(NCCH):
                    halves.append(pv_half(pu, cb, 0))
                    halves.append(pv_half(pu, cb, 1))
            per_round = (len(halves) + NDP - 1) // NDP if halves else 0
            blocks = []
            hi = 0
            for dp in range(NDP):
                blocks.append(scores_round(u, dp))
                for _ in range(per_round):
                    if hi < len(halves):
                        blocks.append(halves[hi])
                        hi += 1
                if dp == 0 and prev_unit is not None:
                    blocks.append(prev_unit.rs_post)
                if fillers:
                    blocks.append(fillers.pop(0))
                    if len(fillers) > 2 * (NDP - 1 - dp):
                        blocks.append(fillers.pop(0))
            while hi < len(halves):
                blocks.append(halves[hi])
                hi += 1
            rsb = rs_block(u)
            blocks.append(rsb[0])
            u.rs_post = rsb[1]
            for pu in prevs:
                blocks.append(ln_finish(pu))
            while fillers:
                blocks.append(fillers.pop(0))
            return blocks

        def drain_unit(u):
            blocks = [u.rs_post]
            for cb in range(NCCH):
                blocks.append(pv_half(u, cb, 0))
                blocks.append(pv_half(u, cb, 1))
                if cb % 2 == 1:
                    blocks.append(ln_finish(u, cb - 1, 2))
            return blocks

        # ---------- schedule ----------
        elems = [make_elem(bi, rep) for rep, bi in enumerate(ELEMS)]

        def load_a_blocks(e):
            return [blk for li in range(NLC) for blk in load_a_chunk(e, li)]

        def elem_apt_blocks(e):
            return [blk for li in range(NLC) for blk in apt_blocks(e, li)]

        # head: critical path Wq+Wk+a0+b0 DMA -> M, q~ proj -> scores.
        e0 = elems[0]
        la = load_a_blocks(e0)
        b_dma0, b_casts0 = load_b_split(e0)
        head = []
        if apply_qkv_bias:
            head += load_weight_blocks("Wq", 0)
            head += [la[0], la[1]]
            head += load_weight_blocks("Wk", 1)
            head += [la[2], la[3]]
            head += b_dma0
            head += qk_proj_blocks(e0, "Wq", "bq", "a8", "qT")
            head += elem_apt_blocks(e0)
            head += b_casts0
            head += qk_proj_blocks(e0, "Wk", "bk", "b8", "kT")
        else:
            mb = load_m_blocks()
            head += mb[:2]
            head += [la[0], la[1]]
            head += mb[2:]
            head += [la[2], la[3]]
            head += b_dma0
            head += qk_proj_blocks(e0, "Wq", "bq", "a8", "qT")
            head += elem_apt_blocks(e0)
            head += b_casts0
        for blk in head:
            blk()

        # unit pipeline.  fillers[ui] rides unit ui's rounds.
        units = []
        for e in elems:
            for p in range(2):
                for j in range(2):
                    units.append((e, p, j))
        NU = len(units)
        fillers = [[] for _ in range(NU)]
        # elem0's v-proj (Wv lands after b0) rides unit 1; unit 0's PV
        # is deferred one unit.
        fillers[1] = load_weight_blocks("Wv", 0) + v_proj_blocks(e0)
        for ei in range(len(elems) - 1):
            en = elems[ei + 1]
            u0 = 4 * ei
            apt = elem_apt_blocks(en)
            b_dma, b_casts = load_b_split(en)
            qb = qk_proj_blocks(en, "Wq", "bq", "a8", "qT")
            kb = (qk_proj_blocks(en, "Wk", "bk", "b8", "kT")
                  if apply_qkv_bias else [])
            fillers[u0 + 0] += load_a_blocks(en) + apt[:8]
            fillers[u0 + 1] += b_dma + apt[8:24]
            fillers[u0 + 2] += apt[24:] + qb[:2]
            fillers[u0 + 3] += qb[2:] + b_casts + kb
            fillers[u0 + 4] += v_proj_blocks(en)

        prevs_list = [[] for _ in range(NU)]
        for ui in range(2, NU):
            prevs_list[ui] = [0, 1] if ui == 2 else [ui - 1]

        unit_objs = []
        for ui, (e, p, j) in enumerate(units):
            u = Unit(e, p, j)
            unit_objs.append(u)
            prevs = [unit_objs[k] for k in prevs_list[ui]]
            prev_unit = unit_objs[ui - 1] if ui > 0 else None
            for blk in unit_rounds(u, prev_unit, prevs, fillers[ui]):
                blk()
        for blk in drain_unit(unit_objs[-1]):
            blk()

    nc.compile()
    return nc


def _get_nc(apply_qkv_bias, apply_gamma_beta, repeat=1):
    key = (apply_qkv_bias, apply_gamma_beta, repeat)
    if key not in _CACHE:
        _CACHE[key] = _build(*key)
    return _CACHE[key]


def _run(inputs, trace=False):
    from concourse import bass_utils

    a = np.ascontiguousarray(np.asarray(inputs["a"], dtype=np.float32))
    b = np.ascontiguousarray(np.asarray(inputs["b"], dtype=np.float32))
    get = lambda n: np.ascontiguousarray(np.asarray(inputs[n], dtype=np.float32))
    Wq, Wk, Wv = get("Wq"), get("Wk"), get("Wv")
    bq, bk, bv = get("bq"), get("bk"), get("bv")
    gamma, beta = get("gamma"), get("beta")

    apply_qkv_bias = bool(np.any(bq) or np.any(bk) or np.any(bv))
    apply_gamma_beta = bool(np.any(gamma != 1.0) or np.any(beta))
    nc = _get_nc(apply_qkv_bias, apply_gamma_beta)

    in_maps = []
    for c in range(NCORE):
        sl = slice(c * NB, (c + 1) * NB)
        in_maps.append({
            "a": np.ascontiguousarray(a[sl]), "b": np.ascontiguousarray(b[sl]),
            "Wq": Wq, "Wk": Wk, "Wv": Wv,
            "bq": bq, "bk": bk, "bv": bv,
            "gamma": gamma, "beta": beta,
        })
    res = bass_utils.run_bass_kernel_spmd(nc, in_maps,
                                          core_ids=list(range(NCORE)),
                                          trace=trace)
    out = np.concatenate(
        [res.results[c]["out"].reshape(NB, L, C) for c in range(NCORE)], axis=0)
    return out, res


def kernel(**inputs):
    out, _ = _run(inputs, trace=False)
    return out


# revision 26
# speedup vs baseline: 1.1444x; 1.0656x over previous
"""CrossDomainAttention TRN2 kernel: 8-core data-parallel over batch.

Reference computation (per batch element, a/b are (L, C) slices):
  ap = a.T (C, L);  q = ap@Wq.T+bq; k,v from b.T
  attn = softmax(q @ k.T / sqrt(L)) (C, C)
  out = LN(attn @ v + ap) over L, returned as the raw (C*L) buffer viewed (L, C)

v6: fp8 (e4m3) DoubleRow with a j-granular 2-deep software pipeline.
Work is cut into 16 "units" (elem x c-chunk-pair x j); each unit's
rounds interleave on the PE: scores(unit, dp) [4 DR mms + 1 exp on Act]
with half-PV-chains of the previous unit and filler blocks (next
element's DMA/casts/transposes/projections), keeping the PE dense so
HAM stays warm and the drain tail is a single unit's 4 PV chains.
Row-sums are ones-lhsT matmuls at unit end (PSUM in the PV pool),
transposed to per-partition columns via tiny PE transposes.  LN uses
the scale-invariant form (out_pre = rowsum*apT + PV) with a batched
Newton rsqrt.  Residual apT is fp16, transposed from an fp16 copy of a
at 1 cyc/row.
"""

import numpy as np

B, L, C = 16, 512, 2048
NCORE = 8
NB = B // NCORE          # batch elements per core
P = 128
F = 512                  # matmul free-dim tile
NLC = L // P             # 4  l/m chunks
NDB = C // P             # 16 d-blocks / c-blocks
NCCH = C // F            # 4  c chunks
NDP = NDB // 2           # 8  d-pairs (DoubleRow)
NLP = NLC // 2           # 2  l/m pairs (DoubleRow)
LN_EPS = 1e-5
RSTD_SEED = 4.77e-4   # ~1/sqrt(mean var') for the scale-invariant LN form
INV_SQRT_L = 1.0 / float(np.sqrt(L))
MSCALE = 64.0   # M = Wq^T Wk is stored as M*MSCALE so fp8e4 stays normal

_CACHE = {}


def _build(apply_qkv_bias: bool, apply_gamma_beta: bool, repeat: int = 1):
    import concourse.bass as bass
    import concourse.tile as tile
    from concourse import bacc, mybir
    from concourse.bass import ts, ds
    from concourse.masks import make_identity
    from contextlib import ExitStack

    f32 = mybir.dt.float32
    f16 = mybir.dt.float16
    f8 = mybir.dt.float8e4
    AF = mybir.ActivationFunctionType
    ALU = mybir.AluOpType
    DR = mybir.MatmulPerfMode.DoubleRow

    nc = bacc.Bacc("TRN2", target_bir_lowering=False, debug=False,
                   enable_asserts=False)

    a_d = nc.dram_tensor("a", (NB, L, C), f32, kind="ExternalInput").ap()
    b_d = nc.dram_tensor("b", (NB, L, C), f32, kind="ExternalInput").ap()
    w_d = {n: nc.dram_tensor(n, (L, L), f32, kind="ExternalInput").ap()
           for n in ("Wq", "Wk", "Wv")}
    bias_d = {n: nc.dram_tensor(n, (L,), f32, kind="ExternalInput").ap()
              for n in ("bq", "bk", "bv")}
    gamma_d = nc.dram_tensor("gamma", (L,), f32, kind="ExternalInput").ap()
    beta_d = nc.dram_tensor("beta", (L,), f32, kind="ExternalInput").ap()
    out_d = nc.dram_tensor("out", (NB, C, L), f32, kind="ExternalOutput").ap()

    def bcast_p(ap1d):
        return bass.AP(tensor=ap1d.tensor, offset=ap1d.offset,
                       ap=[[0, P]] + [list(d) for d in ap1d.ap])

    ELEMS = [i % NB for i in range(NB * repeat)]

    with tile.TileContext(nc) as tc, ExitStack() as ctx:
        const = ctx.enter_context(tc.tile_pool(name="const", bufs=1))
        ld = ctx.enter_context(tc.tile_pool(name="ld", bufs=3))
        c16 = ctx.enter_context(tc.tile_pool(name="c16", bufs=2))
        epool = ctx.enter_context(tc.tile_pool(name="epool", bufs=2))
        pt = ctx.enter_context(tc.tile_pool(name="pt", bufs=4))
        outp = ctx.enter_context(tc.tile_pool(name="outp", bufs=6))
        small = ctx.enter_context(tc.tile_pool(name="small", bufs=2))
        # PSUM: mm 3x[P,2,F](6 banks) + pv 2x[P,F](2) = 8.  The scores
        # pool is 3 deep so exp (Act) never gates the PE; psr lives in an
        # mm slot, trs in a pv slot.
        ps_mm = ctx.enter_context(tc.tile_pool(name="ps_mm", bufs=3, space="PSUM"))
        ps_pv = ctx.enter_context(tc.tile_pool(name="ps_pv", bufs=2, space="PSUM"))

        def cp(e, dst, src):
            if e is nc.scalar:
                e.copy(dst, src)
            else:
                e.tensor_copy(dst, src)

        # ---- constants ----
        ident16 = const.tile([P, P], f16, tag="ident16")
        make_identity(nc, ident16)
        ident1 = const.tile([1, 1], f32, tag="ident1")
        nc.vector.memset(ident1[:], 1.0)
        ones2 = const.tile([P, 2, 16], f8, tag="ones2")
        nc.vector.memset(ones2[:], 1.0)
        bias_col = {}
        bv_bc = None
        if apply_qkv_bias:
            cpack = const.tile([P, 16], f32, tag="cpack")
            for i, n in enumerate(("bq", "bk")):
                dst = cpack[:, 4 * i: 4 * (i + 1)]
                nc.sync.dma_start(dst, bias_d[n].rearrange("(o p) -> p o", p=P))
                bias_col[n] = dst
            bv_bc = const.tile([P, L], f32, tag="bv_bc")
            nc.sync.dma_start(bv_bc[:], bcast_p(bias_d["bv"]))
        if apply_gamma_beta:
            gb_pack = const.tile([P, 2, L], f32, tag="gb")
            nc.sync.dma_start(gb_pack[:, 0, :], bcast_p(gamma_d))
            nc.sync.dma_start(gb_pack[:, 1, :], bcast_p(beta_d))

        wt_names = ("Wq", "Wk", "Wv") if apply_qkv_bias else ("Wv",)
        WT = {n: const.tile([P, NLC, L], f8, tag=f"WT_{n}", name=f"WT_{n}")
              for n in wt_names}
        if not apply_qkv_bias:
            M8 = const.tile([P, NLC, L], f8, tag="M8", name="M8")

        # ---------- emission helpers ----------
        w_eng = [nc.vector, nc.scalar]

        def load_weight_blocks(n, wi):
            def emit():
                wld = ld.tile([P, NLC, F], f32, tag="ld", name=f"wld_{n}")
                nc.sync.dma_start(wld[:],
                                  w_d[n].rearrange("(o p) l -> p o l", p=P))
                w16 = c16.tile([P, NLC, F], f16, tag="w16", bufs=2,
                               name=f"w16_{n}")
                nc.vector.tensor_copy(w16[:], wld[:])
                for li in range(NLC):
                    pst = ps_mm.tile([P, F], f16, tag="mm", name="wtp")
                    for mo in range(NLC):
                        nc.tensor.transpose(pst[:, ts(mo, P)],
                                            w16[:, mo, ts(li, P)], ident16[:])
                    cp(w_eng[(wi + li) % 2], WT[n][:, li, :], pst[:])
            return [emit]

        def load_m_blocks():
            # scores = (a^T (Wq^T Wk)) b: precompute M8 = Wq^T Wk * MSCALE
            # directly in the [l_p, li, l'] weight layout (no transposes).
            w16s = {}

            def ld_one(n):
                def emit():
                    wld = ld.tile([P, NLC, F], f32, tag="ld",
                                  name=f"wld_{n}")
                    nc.sync.dma_start(
                        wld[:], w_d[n].rearrange("(o p) l -> p o l", p=P))
                    w16 = c16.tile([P, NLC, F], f16, tag="w16", bufs=2,
                                   name=f"w16_{n}")
                    nc.vector.tensor_copy(w16[:], wld[:])
                    w16s[n] = w16
                return emit

            def mk_mm(li):
                def emit():
                    ps = ps_mm.tile([P, F], f32, tag="mm", name="mps")
                    for mo in range(NLC):
                        nc.tensor.matmul(
                            ps[:],
                            lhsT=w16s["Wq"][:, mo, ts(li, P)],
                            rhs=w16s["Wk"][:, mo, :],
                            start=(mo == 0), stop=(mo == NLC - 1))
                    nc.scalar.activation(M8[:, li, :], ps[:], AF.Copy,
                                         scale=MSCALE)
                return emit
            return [ld_one("Wq"), ld_one("Wk")] + [mk_mm(li)
                                                   for li in range(NLC)]

        class E:
            pass

        def make_elem(bi, rep):
            e = E()
            e.bi = bi
            e.rep = rep
            e.a8 = epool.tile([P, NLC, C], f8, tag="a8", name=f"a8_{rep}")
            e.b8 = epool.tile([P, NLC, C], f8, tag="b8", name=f"b8_{rep}")
            e.qT = epool.tile([P, NLC, C], f8, tag="qT", name=f"qT_{rep}")
            e.kT = epool.tile([P, NLC, C], f8, tag="kT", name=f"kT_{rep}")
            e.v8 = epool.tile([P, NDB, L], f8, tag="v8", name=f"v8_{rep}")
            e.apT = epool.tile([P, NDB, L], f16, tag="apT", name=f"apT_{rep}")
            e.a16 = {}
            return e

        a_cast8 = [nc.vector, nc.scalar, nc.vector, nc.scalar]
        b_cast8 = [nc.scalar, nc.vector, nc.scalar, nc.vector]
        apt_cp = [nc.vector, nc.scalar, nc.vector, nc.scalar]

        def load_a_chunk(e, li):
            def emit():
                ach = ld.tile([P, C], f32, tag="ld", name=f"a_{e.rep}_{li}")
                nc.sync.dma_start(ach[:], a_d[e.bi, ds(li * P, P), :])
                a16 = c16.tile([P, C], f16, tag="a16", bufs=4,
                               name=f"a16_{e.rep}_{li}")
                nc.vector.tensor_copy(a16[:], ach[:])
                cp(a_cast8[li], e.a8[:, li, :], ach[:])
                e.a16[li] = a16
            return [emit]

        def apt_blocks(e, li):
            # 2 transposes per sub-block; copy after the 2nd sub-block
            state = {}

            def mk(g, h):
                def emit():
                    a16 = e.a16[li]
                    if h == 0:
                        state[g] = ps_mm.tile([P, F], f16, tag="mm",
                                              name="atp")
                    pst = state[g]
                    for j in range(2):
                        db = 4 * g + 2 * h + j
                        nc.tensor.transpose(pst[:, ts(2 * h + j, P)],
                                            a16[:, ts(db, P)], ident16[:])
                    if h == 1:
                        dst = e.apT[:, 4 * g:4 * g + 4, ts(li, P)]
                        cp(apt_cp[(li + g) % 2],
                           dst, pst[:].rearrange("p (b f) -> p b f", f=P))
                return emit
            return [mk(g, h) for g in range(NLC) for h in range(2)]

        def load_b_split(e):
            tiles = {}

            def dma_blk():
                for li in range(NLC):
                    bch = ld.tile([P, C], f32, tag="ld",
                                  name=f"b_{e.rep}_{li}")
                    nc.sync.dma_start(bch[:], b_d[e.bi, ds(li * P, P), :])
                    tiles[li] = bch

            def mk_cast(li):
                def emit():
                    cp(b_cast8[li], e.b8[:, li, :], tiles[li])
                return emit
            return [dma_blk], [mk_cast(li) for li in range(NLC)]

        qk_cp = [nc.vector, nc.scalar]

        def qk_proj_blocks(e, wname, bname, src_name, dst_name):
            def mk(mi):
                def emit():
                    wt = (M8 if (wname == "Wq" and not apply_qkv_bias)
                          else WT[wname])
                    src = getattr(e, src_name)
                    dst = getattr(e, dst_name)
                    pss = [ps_mm.tile([P, F], f32, tag="mm", name=f"qk{i}")
                           for i in range(4)]
                    for lp in range(NLP):
                        for ci in range(NCCH):
                            nc.tensor.matmul(
                                pss[ci][:],
                                lhsT=wt[:, 2 * lp:2 * lp + 2, ts(mi, P)],
                                rhs=src[:, 2 * lp:2 * lp + 2, ts(ci, F)],
                                start=(lp == 0), stop=(lp == NLP - 1),
                                perf_mode=DR)
                    for ci in range(NCCH):
                        dslice = dst[:, mi, ts(ci, F)]
                        if apply_qkv_bias:
                            nc.scalar.activation(
                                dslice, pss[ci][:], AF.Identity,
                                bias=bias_col[bname][:, mi:mi + 1])
                        else:
                            cp(qk_cp[ci % 2], dslice, pss[ci][:])
                return emit
            return [mk(mi) for mi in range(NLC)]

        def v_proj_blocks(e):
            def mk(dp):
                def emit():
                    pss = [ps_mm.tile([P, F], f32, tag="mm", name=f"v{i}")
                           for i in range(2)]
                    for s in range(2):
                        di = 2 * dp + s
                        for lp in range(NLP):
                            nc.tensor.matmul(
                                pss[s][:],
                                lhsT=e.b8[:, 2 * lp:2 * lp + 2, ts(di, P)],
                                rhs=WT["Wv"][:, 2 * lp:2 * lp + 2, :],
                                start=(lp == 0), stop=(lp == NLP - 1),
                                perf_mode=DR)
                    for s in range(2):
                        cp(qk_cp[(dp + s) % 2], e.v8[:, 2 * dp + s, :],
                           pss[s][:])
                        if apply_qkv_bias:
                            nc.vector.tensor_add(e.v8[:, 2 * dp + s, :],
                                                 e.v8[:, 2 * dp + s, :],
                                                 bv_bc[:, :])
                return emit
            return [mk(dp) for dp in range(NDP)]

        # ---------- attention units ----------

        class Unit:
            # one (elem, p, j): a 1024-row slab of the attention output
            def __init__(u, e, p, j):
                u.e, u.p, u.j = e, p, j
                u.PT = pt.tile([P, NDB, F], f8, tag="pt",
                               name=f"pt_{e.rep}_{p}_{j}")
                u.psr = None
                u.rs_post = None
                u.rs_cols = None
                u.stats = small.tile([P, NCCH, 2], f32, tag="stats",
                                     name=f"st_{e.rep}_{p}_{j}")
                u.chain = {}
                u.outs = {}

        def scores_round(u, dp):
            # 4 DR mms -> [P,2,F] psum; 1 exp (Act)
            def emit():
                e = u.e
                lh = e.kT if apply_qkv_bias else e.b8
                esc = INV_SQRT_L if apply_qkv_bias else INV_SQRT_L / MSCALE
                pss = ps_mm.tile([P, 2, F], f32, tag="mm", name="sc")
                for sb in range(2):
                    di = 2 * dp + sb
                    for mp in range(NLP):
                        nc.tensor.matmul(
                            pss[:, sb, :],
                            lhsT=lh[:, 2 * mp:2 * mp + 2, ts(di, P)],
                            rhs=e.qT[:, 2 * mp:2 * mp + 2,
                                     ts(2 * u.p + u.j, F)],
                            start=(mp == 0), stop=(mp == NLP - 1),
                            perf_mode=DR)
                nc.scalar.activation(u.PT[:, 2 * dp:2 * dp + 2, :],
                                     pss[:], AF.Exp, scale=esc)
            return emit

        def rs_block(u):
            # 8 N=512 ones-lhsT mms -> psr row; psrow copy (Act); 4 tiny
            # transposes; copy columns to SBUF.
            def emit_mms():
                u.psr = ps_mm.tile([16, F], f32, tag="mm", name="psr")
                for dp in range(NDP):
                    nc.tensor.matmul(u.psr[:],
                                     lhsT=ones2[:],
                                     rhs=u.PT[:, 2 * dp:2 * dp + 2, :],
                                     start=(dp == 0), stop=(dp == NDP - 1),
                                     perf_mode=DR,
                                     skip_group_check=True)

            def emit_post():
                psrow = small.tile([1, F], f32, tag="rrow", bufs=1,
                                   name="psrow")
                nc.scalar.copy(psrow[:], u.psr[0:1, :])
                trs_ps = ps_pv.tile([P, NCCH], f32, tag="pv", name="trs")
                for cb in range(NCCH):
                    nc.tensor.transpose(trs_ps[:, cb:cb + 1],
                                        psrow[0:1, ts(cb, P)],
                                        ident1[:])
                u.rs_cols = small.tile([P, NCCH], f32, tag="rcol",
                                       name="rs_cols")
                nc.vector.tensor_copy(u.rs_cols[:], trs_ps[:])
            return [emit_mms, emit_post]

        def pv_half(u, cb, h):
            # half of one PV chain: 4 DR mms; on h==1 also stt+bn (DVE)
            def emit():
                e = u.e
                if h == 0:
                    u.chain[cb] = ps_pv.tile([P, L], f32, tag="pv",
                                             name="po")
                po = u.chain[cb]
                for dp in range(4 * h, 4 * h + 4):
                    nc.tensor.matmul(
                        po[:],
                        lhsT=u.PT[:, 2 * dp:2 * dp + 2, ts(cb, P)],
                        rhs=e.v8[:, 2 * dp:2 * dp + 2, :],
                        start=(dp == 0), stop=(dp == NDP - 1),
                        perf_mode=DR)
                if h == 1:
                    gb = (2 * u.p + u.j) * NCCH + cb
                    out_sb = outp.tile([P, L], f16, tag="out", bufs=5,
                                       name="out_sb")
                    nc.vector.scalar_tensor_tensor(
                        out_sb[:], e.apT[:, gb, :],
                        u.rs_cols[:, cb:cb + 1], po[:], ALU.mult, ALU.add)
                    st6 = small.tile([P, 6], f32, tag="st6", name="st6")
                    nc.vector.bn_stats(st6[:], out_sb[:])
                    nc.vector.bn_aggr(u.stats[:, cb, :], st6[:])
                    u.outs[cb] = out_sb
            return emit

        def ln_finish(u, cb0=0, ncb=NCCH):
            # batched Newton rsqrt over ncb cb's; final scale + DMA
            def emit():
                e = u.e
                var_ap = u.stats[:, cb0:cb0 + ncb, 1]
                y = small.tile([P, 2, NCCH], f32, tag="nwt", name="nwt")
                y = y[:, :, :ncb]
                nc.vector.tensor_scalar(y[:, 0, :], var_ap,
                                        -0.5 * RSTD_SEED ** 3,
                                        1.5 * RSTD_SEED,
                                        ALU.mult, ALU.add)
                for _ in range(3):
                    t = y[:, 1, :]
                    nc.vector.tensor_mul(t, y[:, 0, :], y[:, 0, :])
                    nc.vector.tensor_mul(t, t, var_ap)
                    nc.vector.tensor_scalar(t, t, -0.5, 1.5,
                                            ALU.mult, ALU.add)
                    nc.vector.tensor_mul(y[:, 0, :], y[:, 0, :], t)
                for i, cb in enumerate(range(cb0, cb0 + ncb)):
                    gb = (2 * u.p + u.j) * NCCH + cb
                    out_sb = u.outs[cb]
                    out32 = outp.tile([P, L], f32, tag="out32",
                                      bufs=2, name="out32")
                    nc.vector.tensor_scalar(out32[:], out_sb[:],
                                            u.stats[:, cb, 0:1],
                                            y[:, 0, i:i + 1],
                                            ALU.subtract, ALU.mult)
                    if apply_gamma_beta:
                        nc.vector.tensor_mul(out32[:], out32[:],
                                             gb_pack[:, 0, :])
                        nc.vector.tensor_add(out32[:], out32[:],
                                             gb_pack[:, 1, :])
                    nc.sync.dma_start(out_d[e.bi, ds(gb * P, P), :],
                                      out32[:])
            return emit

        def unit_rounds(u, prev_unit, prevs, fillers):
            # 8 rounds of: scores(dp) + prev half-chains + fillers
            halves = []
            for pu in prevs:
                for cb in range(NCCH):
                    halves.append(pv_half(pu, cb, 0))
                    halves.append(pv_half(pu, cb, 1))
            per_round = (len(halves) + NDP - 1) // NDP if halves else 0
            blocks = []
            hi = 0
            for dp in range(NDP):
                blocks.append(scores_round(u, dp))
                for _ in range(per_round):
                    if hi < len(halves):
                        blocks.append(halves[hi])
                        hi += 1
                if dp == 0 and prev_unit is not None:
                    blocks.append(prev_unit.rs_post)
                if fillers:
                    blocks.append(fillers.pop(0))
                    if len(fillers) > 2 * (NDP - 1 - dp):
                        blocks.append(fillers.pop(0))
            while hi < len(halves):
                blocks.append(halves[hi])
                hi += 1
            rsb = rs_block(u)
            blocks.append(rsb[0])
            u.rs_post = rsb[1]
            for pu in prevs:
                blocks.append(ln_finish(pu))
            while fillers:
                blocks.append(fillers.pop(0))
            return blocks

        def drain_unit(u):
            blocks = [u.rs_post]
            for cb in range(NCCH):
                blocks.append(pv_half(u, cb, 0))
                blocks.append(pv_half(u, cb, 1))
                if cb % 2 == 1:
                    blocks.append(ln_finish(u, cb - 1, 2))
            return blocks

        # ---------- schedule ----------
        elems = [make_elem(bi, rep) for rep, bi in enumerate(ELEMS)]

        def load_a_blocks(e):
            return [blk for li in range(NLC) for blk in load_a_chunk(e, li)]

        def elem_apt_blocks(e):
            return [blk for li in range(NLC) for blk in apt_blocks(e, li)]

        # head: critical path Wq+Wk+a0+b0 DMA -> M, q~ proj -> scores.
        e0 = elems[0]
        la = load_a_blocks(e0)
        b_dma0, b_casts0 = load_b_split(e0)
        head = []
        if apply_qkv_bias:
            head += load_weight_blocks("Wq", 0)
            head += [la[0], la[1]]
            head += load_weight_blocks("Wk", 1)
            head += [la[2], la[3]]
            head += b_dma0
            head += qk_proj_blocks(e0, "Wq", "bq", "a8", "qT")
            head += elem_apt_blocks(e0)
            head += b_casts0
            head += qk_proj_blocks(e0, "Wk", "bk", "b8", "kT")
        else:
            mb = load_m_blocks()
            head += mb[:2]
            head += [la[0], la[1]]
            head += mb[2:]
            head += [la[2], la[3]]
            head += b_dma0
            head += qk_proj_blocks(e0, "Wq", "bq", "a8", "qT")
            head += elem_apt_blocks(e0)
            head += b_casts0
            if len(elems) > 1:
                head += load_a_blocks(elems[1])
        for blk in head:
            blk()

        # unit pipeline.  fillers[ui] rides unit ui's rounds.
        units = []
        for e in elems:
            for p in range(2):
                for j in range(2):
                    units.append((e, p, j))
        NU = len(units)
        fillers = [[] for _ in range(NU)]
        # elem0's v-proj (Wv lands after b0) rides unit 1; unit 0's PV
        # is deferred one unit.
        fillers[1] = load_weight_blocks("Wv", 0) + v_proj_blocks(e0)
        for ei in range(len(elems) - 1):
            en = elems[ei + 1]
            u0 = 4 * ei
            apt = elem_apt_blocks(en)
            b_dma, b_casts = load_b_split(en)
            qb = qk_proj_blocks(en, "Wq", "bq", "a8", "qT")
            kb = (qk_proj_blocks(en, "Wk", "bk", "b8", "kT")
                  if apply_qkv_bias else [])
            if ei > 0 or apply_qkv_bias:
                fillers[u0 + 0] += load_a_blocks(en)
            fillers[u0 + 0] += apt[:8]
            fillers[u0 + 1] += b_dma + apt[8:24]
            fillers[u0 + 2] += apt[24:] + qb[:2]
            fillers[u0 + 3] += qb[2:] + b_casts + kb
            fillers[u0 + 4] += v_proj_blocks(en)

        prevs_list = [[] for _ in range(NU)]
        for ui in range(2, NU):
            prevs_list[ui] = [0, 1] if ui == 2 else [ui - 1]

        unit_objs = []
        for ui, (e, p, j) in enumerate(units):
            u = Unit(e, p, j)
            unit_objs.append(u)
            prevs = [unit_objs[k] for k in prevs_list[ui]]
            prev_unit = unit_objs[ui - 1] if ui > 0 else None
            for blk in unit_rounds(u, prev_unit, prevs, fillers[ui]):
                blk()
        for blk in drain_unit(unit_objs[-1]):
            blk()

    nc.compile()
    return nc


def _get_nc(apply_qkv_bias, apply_gamma_beta, repeat=1):
    key = (apply_qkv_bias, apply_gamma_beta, repeat)
    if key not in _CACHE:
        _CACHE[key] = _build(*key)
    return _CACHE[key]


def _run(inputs, trace=False):
    from concourse import bass_utils

    a = np.ascontiguousarray(np.asarray(inputs["a"], dtype=np.float32))
    b = np.ascontiguousarray(np.asarray(inputs["b"], dtype=np.float32))
    get = lambda n: np.ascontiguousarray(np.asarray(inputs[n], dtype=np.float32))
    Wq, Wk, Wv = get("Wq"), get("Wk"), get("Wv")
    bq, bk, bv = get("bq"), get("bk"), get("bv")
    gamma, beta = get("gamma"), get("beta")

    apply_qkv_bias = bool(np.any(bq) or np.any(bk) or np.any(bv))
    apply_gamma_beta = bool(np.any(gamma != 1.0) or np.any(beta))
    nc = _get_nc(apply_qkv_bias, apply_gamma_beta)

    in_maps = []
    for c in range(NCORE):
        sl = slice(c * NB, (c + 1) * NB)
        in_maps.append({
            "a": np.ascontiguousarray(a[sl]), "b": np.ascontiguousarray(b[sl]),
            "Wq": Wq, "Wk": Wk, "Wv": Wv,
            "bq": bq, "bk": bk, "bv": bv,
            "gamma": gamma, "beta": beta,
        })
    res = bass_utils.run_bass_kernel_spmd(nc, in_maps,
                                          core_ids=list(range(NCORE)),
                                          trace=trace)
    out = np.concatenate(
        [res.results[c]["out"].reshape(NB, L, C) for c in range(NCORE)], axis=0)
    return out, res


def kernel(**inputs):
    out, _ = _run(inputs, trace=False)
    return out


# revision 29
# speedup vs baseline: 1.1517x; 1.0064x over previous
"""CrossDomainAttention TRN2 kernel: 8-core data-parallel over batch.

Reference computation (per batch element, a/b are (L, C) slices):
  ap = a.T (C, L);  q = ap@Wq.T+bq; k,v from b.T
  attn = softmax(q @ k.T / sqrt(L)) (C, C)
  out = LN(attn @ v + ap) over L, returned as the raw (C*L) buffer viewed (L, C)

v6: fp8 (e4m3) DoubleRow with a j-granular 2-deep software pipeline.
Work is cut into 16 "units" (elem x c-chunk-pair x j); each unit's
rounds interleave on the PE: scores(unit, dp) [4 DR mms + 1 exp on Act]
with half-PV-chains of the previous unit and filler blocks (next
element's DMA/casts/transposes/projections), keeping the PE dense so
HAM stays warm and the drain tail is a single unit's 4 PV chains.
Row-sums are ones-lhsT matmuls at unit end (PSUM in the PV pool),
transposed to per-partition columns via tiny PE transposes.  LN uses
the scale-invariant form (out_pre = rowsum*apT + PV) with a batched
Newton rsqrt.  Residual apT is fp16, transposed from an fp16 copy of a
at 1 cyc/row.
"""

import numpy as np

B, L, C = 16, 512, 2048
NCORE = 8
NB = B // NCORE          # batch elements per core
P = 128
F = 512                  # matmul free-dim tile
NLC = L // P             # 4  l/m chunks
NDB = C // P             # 16 d-blocks / c-blocks
NCCH = C // F            # 4  c chunks
NDP = NDB // 2           # 8  d-pairs (DoubleRow)
NLP = NLC // 2           # 2  l/m pairs (DoubleRow)
LN_EPS = 1e-5
RSTD_SEED = 4.77e-4   # ~1/sqrt(mean var') for the scale-invariant LN form
INV_SQRT_L = 1.0 / float(np.sqrt(L))
MSCALE = 64.0   # M = Wq^T Wk is stored as M*MSCALE so fp8e4 stays normal

_CACHE = {}


def _build(apply_qkv_bias: bool, apply_gamma_beta: bool, repeat: int = 1):
    import concourse.bass as bass
    import concourse.tile as tile
    from concourse import bacc, mybir
    from concourse.bass import ts, ds
    from concourse.masks import make_identity
    from contextlib import ExitStack

    f32 = mybir.dt.float32
    f16 = mybir.dt.float16
    f8 = mybir.dt.float8e4
    AF = mybir.ActivationFunctionType
    ALU = mybir.AluOpType
    DR = mybir.MatmulPerfMode.DoubleRow

    nc = bacc.Bacc("TRN2", target_bir_lowering=False, debug=False,
                   enable_asserts=False)

    a_d = nc.dram_tensor("a", (NB, L, C), f32, kind="ExternalInput").ap()
    b_d = nc.dram_tensor("b", (NB, L, C), f32, kind="ExternalInput").ap()
    w_d = {n: nc.dram_tensor(n, (L, L), f32, kind="ExternalInput").ap()
           for n in ("Wq", "Wk", "Wv")}
    bias_d = {n: nc.dram_tensor(n, (L,), f32, kind="ExternalInput").ap()
              for n in ("bq", "bk", "bv")}
    gamma_d = nc.dram_tensor("gamma", (L,), f32, kind="ExternalInput").ap()
    beta_d = nc.dram_tensor("beta", (L,), f32, kind="ExternalInput").ap()
    out_d = nc.dram_tensor("out", (NB, C, L), f32, kind="ExternalOutput").ap()

    def bcast_p(ap1d):
        return bass.AP(tensor=ap1d.tensor, offset=ap1d.offset,
                       ap=[[0, P]] + [list(d) for d in ap1d.ap])

    ELEMS = [i % NB for i in range(NB * repeat)]

    with tile.TileContext(nc) as tc, ExitStack() as ctx:
        const = ctx.enter_context(tc.tile_pool(name="const", bufs=1))
        ld = ctx.enter_context(tc.tile_pool(name="ld", bufs=3))
        c16 = ctx.enter_context(tc.tile_pool(name="c16", bufs=2))
        epool = ctx.enter_context(tc.tile_pool(name="epool", bufs=2))
        pt = ctx.enter_context(tc.tile_pool(name="pt", bufs=4))
        outp = ctx.enter_context(tc.tile_pool(name="outp", bufs=6))
        small = ctx.enter_context(tc.tile_pool(name="small", bufs=2))
        # PSUM: mm 3x[P,2,F](6 banks) + pv 2x[P,F](2) = 8.  The scores
        # pool is 3 deep so exp (Act) never gates the PE; psr lives in an
        # mm slot, trs in a pv slot.
        ps_mm = ctx.enter_context(tc.tile_pool(name="ps_mm", bufs=3, space="PSUM"))
        ps_pv = ctx.enter_context(tc.tile_pool(name="ps_pv", bufs=2, space="PSUM"))

        def cp(e, dst, src):
            if e is nc.scalar:
                e.copy(dst, src)
            else:
                e.tensor_copy(dst, src)

        # ---- constants ----
        ident16 = const.tile([P, P], f16, tag="ident16")
        make_identity(nc, ident16)
        ident1 = const.tile([1, 1], f32, tag="ident1")
        nc.vector.memset(ident1[:], 1.0)
        ones2 = const.tile([P, 2, 16], f8, tag="ones2")
        nc.vector.memset(ones2[:], 1.0)
        bias_col = {}
        bv_bc = None
        if apply_qkv_bias:
            cpack = const.tile([P, 16], f32, tag="cpack")
            for i, n in enumerate(("bq", "bk")):
                dst = cpack[:, 4 * i: 4 * (i + 1)]
                nc.sync.dma_start(dst, bias_d[n].rearrange("(o p) -> p o", p=P))
                bias_col[n] = dst
            bv_bc = const.tile([P, L], f32, tag="bv_bc")
            nc.sync.dma_start(bv_bc[:], bcast_p(bias_d["bv"]))
        if apply_gamma_beta:
            gb_pack = const.tile([P, 2, L], f32, tag="gb")
            nc.sync.dma_start(gb_pack[:, 0, :], bcast_p(gamma_d))
            nc.sync.dma_start(gb_pack[:, 1, :], bcast_p(beta_d))

        wt_names = ("Wq", "Wk", "Wv") if apply_qkv_bias else ("Wv",)
        WT = {n: const.tile([P, NLC, L], f8, tag=f"WT_{n}", name=f"WT_{n}")
              for n in wt_names}
        if not apply_qkv_bias:
            M8 = const.tile([P, NLC, L], f8, tag="M8", name="M8")

        # ---------- emission helpers ----------
        w_eng = [nc.vector, nc.scalar]

        def load_weight_blocks(n, wi):
            def emit():
                wld = ld.tile([P, NLC, F], f32, tag="ld", name=f"wld_{n}")
                nc.sync.dma_start(wld[:],
                                  w_d[n].rearrange("(o p) l -> p o l", p=P))
                w16 = c16.tile([P, NLC, F], f16, tag="w16", bufs=2,
                               name=f"w16_{n}")
                nc.vector.tensor_copy(w16[:], wld[:])
                for li in range(NLC):
                    pst = ps_mm.tile([P, F], f16, tag="mm", name="wtp")
                    for mo in range(NLC):
                        nc.tensor.transpose(pst[:, ts(mo, P)],
                                            w16[:, mo, ts(li, P)], ident16[:])
                    cp(w_eng[(wi + li) % 2], WT[n][:, li, :], pst[:])
            return [emit]

        def load_m_blocks():
            # scores = (a^T (Wq^T Wk)) b: precompute M8 = Wq^T Wk * MSCALE
            # directly in the [l_p, li, l'] weight layout (no transposes).
            w16s = {}

            def ld_one(n):
                def emit():
                    wld = ld.tile([P, NLC, F], f32, tag="ld",
                                  name=f"wld_{n}")
                    nc.sync.dma_start(
                        wld[:], w_d[n].rearrange("(o p) l -> p o l", p=P))
                    w16 = c16.tile([P, NLC, F], f16, tag="w16", bufs=2,
                                   name=f"w16_{n}")
                    nc.vector.tensor_copy(w16[:], wld[:])
                    w16s[n] = w16
                return emit

            def mk_mm(li):
                def emit():
                    ps = ps_mm.tile([P, F], f32, tag="mm", name="mps")
                    for mo in range(NLC):
                        nc.tensor.matmul(
                            ps[:],
                            lhsT=w16s["Wq"][:, mo, ts(li, P)],
                            rhs=w16s["Wk"][:, mo, :],
                            start=(mo == 0), stop=(mo == NLC - 1))
                    nc.scalar.activation(M8[:, li, :], ps[:], AF.Copy,
                                         scale=MSCALE)
                return emit
            return [ld_one("Wq"), ld_one("Wk")] + [mk_mm(li)
                                                   for li in range(NLC)]

        class E:
            pass

        def make_elem(bi, rep):
            e = E()
            e.bi = bi
            e.rep = rep
            e.a8 = epool.tile([P, NLC, C], f8, tag="a8", name=f"a8_{rep}")
            e.b8 = epool.tile([P, NLC, C], f8, tag="b8", name=f"b8_{rep}")
            e.qT = epool.tile([P, NLC, C], f8, tag="qT", name=f"qT_{rep}")
            e.kT = epool.tile([P, NLC, C], f8, tag="kT", name=f"kT_{rep}")
            e.v8 = epool.tile([P, NDB, L], f8, tag="v8", name=f"v8_{rep}")
            e.apT = epool.tile([P, NDB, L], f16, tag="apT", name=f"apT_{rep}")
            e.a16 = {}
            return e

        a_cast8 = [nc.vector, nc.scalar, nc.vector, nc.scalar]
        b_cast8 = [nc.scalar, nc.vector, nc.scalar, nc.vector]
        apt_cp = [nc.vector, nc.scalar, nc.vector, nc.scalar]

        def load_a_chunk(e, li):
            def emit():
                ach = ld.tile([P, C], f32, tag="ld", name=f"a_{e.rep}_{li}")
                nc.sync.dma_start(ach[:], a_d[e.bi, ds(li * P, P), :])
                a16 = c16.tile([P, C], f16, tag="a16", bufs=4,
                               name=f"a16_{e.rep}_{li}")
                nc.vector.tensor_copy(a16[:], ach[:])
                cp(a_cast8[li], e.a8[:, li, :], ach[:])
                e.a16[li] = a16
            return [emit]

        def apt_blocks(e, li):
            # 2 transposes per sub-block; copy after the 2nd sub-block
            state = {}

            def mk(g, h):
                def emit():
                    a16 = e.a16[li]
                    if h == 0:
                        state[g] = ps_mm.tile([P, F], f16, tag="mm",
                                              name="atp")
                    pst = state[g]
                    for j in range(2):
                        db = 4 * g + 2 * h + j
                        nc.tensor.transpose(pst[:, ts(2 * h + j, P)],
                                            a16[:, ts(db, P)], ident16[:])
                    if h == 1:
                        dst = e.apT[:, 4 * g:4 * g + 4, ts(li, P)]
                        cp(apt_cp[(li + g) % 2],
                           dst, pst[:].rearrange("p (b f) -> p b f", f=P))
                return emit
            return [mk(g, h) for g in range(NLC) for h in range(2)]

        def load_b_split(e):
            tiles = {}

            def dma_blk():
                for li in range(NLC):
                    bch = ld.tile([P, C], f32, tag="ld",
                                  name=f"b_{e.rep}_{li}")
                    nc.sync.dma_start(bch[:], b_d[e.bi, ds(li * P, P), :])
                    tiles[li] = bch

            def mk_cast(li):
                def emit():
                    cp(b_cast8[li], e.b8[:, li, :], tiles[li])
                return emit
            return [dma_blk], [mk_cast(li) for li in range(NLC)]

        qk_cp = [nc.vector, nc.scalar]

        def qk_proj_blocks(e, wname, bname, src_name, dst_name):
            def mk(mi):
                def emit():
                    wt = (M8 if (wname == "Wq" and not apply_qkv_bias)
                          else WT[wname])
                    src = getattr(e, src_name)
                    dst = getattr(e, dst_name)
                    pss = [ps_mm.tile([P, F], f32, tag="mm", name=f"qk{i}")
                           for i in range(4)]
                    for lp in range(NLP):
                        for ci in range(NCCH):
                            nc.tensor.matmul(
                                pss[ci][:],
                                lhsT=wt[:, 2 * lp:2 * lp + 2, ts(mi, P)],
                                rhs=src[:, 2 * lp:2 * lp + 2, ts(ci, F)],
                                start=(lp == 0), stop=(lp == NLP - 1),
                                perf_mode=DR)
                    for ci in range(NCCH):
                        dslice = dst[:, mi, ts(ci, F)]
                        if apply_qkv_bias:
                            nc.scalar.activation(
                                dslice, pss[ci][:], AF.Identity,
                                bias=bias_col[bname][:, mi:mi + 1])
                        else:
                            cp(qk_cp[ci % 2], dslice, pss[ci][:])
                return emit
            return [mk(mi) for mi in range(NLC)]

        def v_proj_blocks(e):
            def mk(dp):
                def emit():
                    pss = [ps_mm.tile([P, F], f32, tag="mm", name=f"v{i}")
                           for i in range(2)]
                    for s in range(2):
                        di = 2 * dp + s
                        for lp in range(NLP):
                            nc.tensor.matmul(
                                pss[s][:],
                                lhsT=e.b8[:, 2 * lp:2 * lp + 2, ts(di, P)],
                                rhs=WT["Wv"][:, 2 * lp:2 * lp + 2, :],
                                start=(lp == 0), stop=(lp == NLP - 1),
                                perf_mode=DR)
                    for s in range(2):
                        cp(qk_cp[(dp + s) % 2], e.v8[:, 2 * dp + s, :],
                           pss[s][:])
                        if apply_qkv_bias:
                            nc.vector.tensor_add(e.v8[:, 2 * dp + s, :],
                                                 e.v8[:, 2 * dp + s, :],
                                                 bv_bc[:, :])
                return emit
            return [mk(dp) for dp in range(NDP)]

        # ---------- attention units ----------

        class Unit:
            # one (elem, p, j): a 1024-row slab of the attention output
            def __init__(u, e, p, j):
                u.e, u.p, u.j = e, p, j
                u.PT = pt.tile([P, NDB, F], f8, tag="pt",
                               name=f"pt_{e.rep}_{p}_{j}")
                u.psr = None
                u.rs_post = None
                u.rs_cols = None
                u.stats = small.tile([P, NCCH, 2], f32, tag="stats",
                                     name=f"st_{e.rep}_{p}_{j}")
                u.chain = {}
                u.outs = {}

        def scores_round(u, dp):
            # 4 DR mms -> [P,2,F] psum; 1 exp (Act)
            def emit():
                e = u.e
                lh = e.kT if apply_qkv_bias else e.b8
                esc = INV_SQRT_L if apply_qkv_bias else INV_SQRT_L / MSCALE
                pss = ps_mm.tile([P, 2, F], f32, tag="mm", name="sc")
                for sb in range(2):
                    di = 2 * dp + sb
                    for mp in range(NLP):
                        nc.tensor.matmul(
                            pss[:, sb, :],
                            lhsT=lh[:, 2 * mp:2 * mp + 2, ts(di, P)],
                            rhs=e.qT[:, 2 * mp:2 * mp + 2,
                                     ts(2 * u.p + u.j, F)],
                            start=(mp == 0), stop=(mp == NLP - 1),
                            perf_mode=DR)
                nc.scalar.activation(u.PT[:, 2 * dp:2 * dp + 2, :],
                                     pss[:], AF.Exp, scale=esc)
            return emit

        def rs_block(u):
            # 8 N=512 ones-lhsT mms -> psr row; psrow copy (Act); 4 tiny
            # transposes; copy columns to SBUF.
            def emit_mms():
                u.psr = ps_mm.tile([16, F], f32, tag="mm", name="psr")
                for dp in range(NDP):
                    nc.tensor.matmul(u.psr[:],
                                     lhsT=ones2[:],
                                     rhs=u.PT[:, 2 * dp:2 * dp + 2, :],
                                     start=(dp == 0), stop=(dp == NDP - 1),
                                     perf_mode=DR,
                                     skip_group_check=True)

            def emit_post():
                psrow = small.tile([1, F], f32, tag="rrow", bufs=1,
                                   name="psrow")
                nc.scalar.copy(psrow[:], u.psr[0:1, :])
                trs_ps = ps_pv.tile([P, NCCH], f32, tag="pv", name="trs")
                for cb in range(NCCH):
                    nc.tensor.transpose(trs_ps[:, cb:cb + 1],
                                        psrow[0:1, ts(cb, P)],
                                        ident1[:])
                u.rs_cols = small.tile([P, NCCH], f32, tag="rcol",
                                       name="rs_cols")
                nc.vector.tensor_copy(u.rs_cols[:], trs_ps[:])
            return [emit_mms, emit_post]

        def pv_half(u, cb, h):
            # half of one PV chain: 4 DR mms; on h==1 also stt+bn (DVE)
            def emit():
                e = u.e
                if h == 0:
                    u.chain[cb] = ps_pv.tile([P, L], f32, tag="pv",
                                             name="po")
                po = u.chain[cb]
                for dp in range(4 * h, 4 * h + 4):
                    nc.tensor.matmul(
                        po[:],
                        lhsT=u.PT[:, 2 * dp:2 * dp + 2, ts(cb, P)],
                        rhs=e.v8[:, 2 * dp:2 * dp + 2, :],
                        start=(dp == 0), stop=(dp == NDP - 1),
                        perf_mode=DR)
                if h == 1:
                    gb = (2 * u.p + u.j) * NCCH + cb
                    out_sb = outp.tile([P, L], f16, tag="out", bufs=5,
                                       name="out_sb")
                    nc.vector.scalar_tensor_tensor(
                        out_sb[:], e.apT[:, gb, :],
                        u.rs_cols[:, cb:cb + 1], po[:], ALU.mult, ALU.add)
                    st6 = small.tile([P, 6], f32, tag="st6", name="st6")
                    nc.vector.bn_stats(st6[:], out_sb[:])
                    nc.vector.bn_aggr(u.stats[:, cb, :], st6[:])
                    u.outs[cb] = out_sb
            return emit

        def ln_finish(u, cb0=0, ncb=NCCH):
            # batched Newton rsqrt over ncb cb's; final scale + DMA
            def emit():
                e = u.e
                var_ap = u.stats[:, cb0:cb0 + ncb, 1]
                y = small.tile([P, 2, NCCH], f32, tag="nwt", name="nwt")
                y = y[:, :, :ncb]
                nc.vector.tensor_scalar(y[:, 0, :], var_ap,
                                        -0.5 * RSTD_SEED ** 3,
                                        1.5 * RSTD_SEED,
                                        ALU.mult, ALU.add)
                for _ in range(3):
                    t = y[:, 1, :]
                    nc.vector.tensor_mul(t, y[:, 0, :], y[:, 0, :])
                    nc.vector.tensor_mul(t, t, var_ap)
                    nc.vector.tensor_scalar(t, t, -0.5, 1.5,
                                            ALU.mult, ALU.add)
                    nc.vector.tensor_mul(y[:, 0, :], y[:, 0, :], t)
                for i, cb in enumerate(range(cb0, cb0 + ncb)):
                    gb = (2 * u.p + u.j) * NCCH + cb
                    out_sb = u.outs[cb]
                    out32 = outp.tile([P, L], f32, tag="out32",
                                      bufs=2, name="out32")
                    nc.vector.tensor_scalar(out32[:], out_sb[:],
                                            u.stats[:, cb, 0:1],
                                            y[:, 0, i:i + 1],
                                            ALU.subtract, ALU.mult)
                    if apply_gamma_beta:
                        nc.vector.tensor_mul(out32[:], out32[:],
                                             gb_pack[:, 0, :])
                        nc.vector.tensor_add(out32[:], out32[:],
                                             gb_pack[:, 1, :])
                    nc.sync.dma_start(out_d[e.bi, ds(gb * P, P), :],
                                      out32[:])
            return emit

        def unit_rounds(u, prev_unit, prevs, fillers):
            # 8 rounds of: scores(dp) + prev half-chains + fillers
            halves = []
            for pu in prevs:
                for cb in range(NCCH):
                    halves.append(pv_half(pu, cb, 0))
                    halves.append(pv_half(pu, cb, 1))
            per_round = (len(halves) + NDP - 1) // NDP if halves else 0
            blocks = []
            hi = 0
            for dp in range(NDP):
                blocks.append(scores_round(u, dp))
                for _ in range(per_round):
                    if hi < len(halves):
                        blocks.append(halves[hi])
                        hi += 1
                if dp == 0 and prev_unit is not None:
                    blocks.append(prev_unit.rs_post)
                if fillers:
                    blocks.append(fillers.pop(0))
                    if len(fillers) > 2 * (NDP - 1 - dp):
                        blocks.append(fillers.pop(0))
            while hi < len(halves):
                blocks.append(halves[hi])
                hi += 1
            rsb = rs_block(u)
            blocks.append(rsb[0])
            u.rs_post = rsb[1]
            for pu in prevs:
                blocks.append(ln_finish(pu))
            while fillers:
                blocks.append(fillers.pop(0))
            return blocks

        def drain_unit(u):
            blocks = [u.rs_post]
            for cb in range(NCCH):
                blocks.append(pv_half(u, cb, 0))
                blocks.append(pv_half(u, cb, 1))
                if cb % 2 == 1:
                    blocks.append(ln_finish(u, cb - 1, 2))
            return blocks

        # ---------- schedule ----------
        elems = [make_elem(bi, rep) for rep, bi in enumerate(ELEMS)]

        def load_a_blocks(e):
            return [blk for li in range(NLC) for blk in load_a_chunk(e, li)]

        def elem_apt_blocks(e):
            return [blk for li in range(NLC) for blk in apt_blocks(e, li)]

        # head: critical path Wq+Wk+a0+b0 DMA -> M, q~ proj -> scores.
        e0 = elems[0]
        la = load_a_blocks(e0)
        b_dma0, b_casts0 = load_b_split(e0)
        head = []
        if apply_qkv_bias:
            head += load_weight_blocks("Wq", 0)
            head += [la[0], la[1]]
            head += load_weight_blocks("Wk", 1)
            head += [la[2], la[3]]
            head += b_dma0
            head += qk_proj_blocks(e0, "Wq", "bq", "a8", "qT")
            head += elem_apt_blocks(e0)
            head += b_casts0
            head += qk_proj_blocks(e0, "Wk", "bk", "b8", "kT")
        else:
            mb = load_m_blocks()
            head += mb[:2]
            head += [la[0], la[1]]
            head += mb[2:]
            head += [la[2], la[3]]
            head += b_dma0
            head += qk_proj_blocks(e0, "Wq", "bq", "a8", "qT")
            head += elem_apt_blocks(e0)
            head += b_casts0
            if len(elems) > 1:
                head += load_a_blocks(elems[1])
        for blk in head:
            blk()

        # unit pipeline.  fillers[ui] rides unit ui's rounds.
        units = []
        for e in elems:
            for p in range(2):
                for j in range(2):
                    units.append((e, p, j))
        NU = len(units)
        fillers = [[] for _ in range(NU)]
        # elem0's v-proj (Wv lands after b0) rides unit 1; unit 0's PV
        # is deferred one unit.
        fillers[1] = load_weight_blocks("Wv", 0) + v_proj_blocks(e0)
        for ei in range(len(elems) - 1):
            en = elems[ei + 1]
            u0 = 4 * ei
            apt = elem_apt_blocks(en)
            b_dma, b_casts = load_b_split(en)
            qb = qk_proj_blocks(en, "Wq", "bq", "a8", "qT")
            kb = (qk_proj_blocks(en, "Wk", "bk", "b8", "kT")
                  if apply_qkv_bias else [])
            if ei > 0 or apply_qkv_bias:
                fillers[u0 + 0] += load_a_blocks(en)
            fillers[u0 + 0] += apt[:8]
            fillers[u0 + 1] += b_dma + apt[8:24]
            fillers[u0 + 2] += apt[24:] + qb[:2]
            fillers[u0 + 3] += qb[2:] + b_casts + kb
            fillers[u0 + 4] += v_proj_blocks(en)

        prevs_list = [[] for _ in range(NU)]
        for ui in range(2, NU):
            prevs_list[ui] = [0, 1] if ui == 2 else [ui - 1]

        unit_objs = []
        for ui, (e, p, j) in enumerate(units):
            u = Unit(e, p, j)
            unit_objs.append(u)
            prevs = [unit_objs[k] for k in prevs_list[ui]]
            prev_unit = unit_objs[ui - 1] if ui > 0 else None
            for blk in unit_rounds(u, prev_unit, prevs, fillers[ui]):
                blk()
        for blk in drain_unit(unit_objs[-1]):
            blk()

    nc.compile()
    return nc


def _get_nc(apply_qkv_bias, apply_gamma_beta, repeat=1):
    key = (apply_qkv_bias, apply_gamma_beta, repeat)
    if key not in _CACHE:
        _CACHE[key] = _build(*key)
    return _CACHE[key]


def _run(inputs, trace=False):
    from concourse import bass_utils

    a = np.ascontiguousarray(np.asarray(inputs["a"], dtype=np.float32))
    b = np.ascontiguousarray(np.asarray(inputs["b"], dtype=np.float32))
    get = lambda n: np.ascontiguousarray(np.asarray(inputs[n], dtype=np.float32))
    Wq, Wk, Wv = get("Wq"), get("Wk"), get("Wv")
    bq, bk, bv = get("bq"), get("bk"), get("bv")
    gamma, beta = get("gamma"), get("beta")

    apply_qkv_bias = bool(np.any(bq) or np.any(bk) or np.any(bv))
    apply_gamma_beta = bool(np.any(gamma != 1.0) or np.any(beta))
    nc = _get_nc(apply_qkv_bias, apply_gamma_beta)

    in_maps = []
    for c in range(NCORE):
        sl = slice(c * NB, (c + 1) * NB)
        in_maps.append({
            "a": np.ascontiguousarray(a[sl]), "b": np.ascontiguousarray(b[sl]),
            "Wq": Wq, "Wk": Wk, "Wv": Wv,
            "bq": bq, "bk": bk, "bv": bv,
            "gamma": gamma, "beta": beta,
        })
    res = bass_utils.run_bass_kernel_spmd(nc, in_maps,
                                          core_ids=list(range(NCORE)),
                                          trace=trace)
    out = np.concatenate(
        [res.results[c]["out"].reshape(NB, L, C) for c in range(NCORE)], axis=0)
    return out, res


def kernel(**inputs):
    out, _ = _run(inputs, trace=False)
    return out
